# revision 1
# baseline (speedup 1.0000x reference)
"""Trainium2 Bass kernel for nn_MultiHeadAttention_66202625900642.

Reference semantics (B=2, S=2048, E=1024, H=16 heads, D=64):
    qh = q @ Wq.T + bq   (same k, v)
    head split is a PLAIN RESHAPE (B, S, E) -> (B, H, S, D):
      head h of batch b = rows [128h, 128h+128) of qh[b] reinterpreted
      row-major as a (2048, 64) matrix (scrambled seq index s' = 16r + c).
    causal softmax over s', out @ Wp.T + bp.

Because the head split partitions the *sequence* rows, sharding each batch
into 4 row-blocks of 512 (= 4 heads) is fully local: 8 cores = 2 batches x 4
quarters, zero collectives. Weights are replicated (fp16).

Per-core pipeline (all matmuls fp16, fp32 PSUM accumulation):
  1. projections -> qh/kh/vh fp16 (weights streamed per e-tile)
  2. DRAM round-trip: qh/kh into per-pair [2048, 128] files (2 heads wide),
     vh natural; DMA-transpose pair files back as [128, 2048] = two heads'
     Q_hT/K_hT stacked; vh re-read as [128, 65] V' tiles (ones column -> row
     sums ride along the P^T @ V' matmul).
  3. attention per head pair, both heads' S^T blocks issued to disjoint PE
     row groups (K=64 each -> concurrent on the 128x128 array): one exp per
     psum group on ACT, causal triangles via gpsimd affine_select,
     P^T @ V' accumulates out^T[d, s'] + rowsum into per-chunk PSUM,
     evacuated to SBUF.
  4. normalization: reciprocal of rowsum, PE-broadcast (f32r K=1 matmul),
     fused into the stride-16 rearrange to final-projection layout.
  5. final projection -> y fp32.
"""

import numpy as np

import concourse.bass as bass
import concourse.mybir as mybir
import concourse.tile as tile
from concourse import bacc
from concourse.bass_utils import run_bass_kernel_spmd

F16 = mybir.dt.float16
F32 = mybir.dt.float32
F32R = mybir.dt.float32r
EXP = mybir.ActivationFunctionType.Exp

B, S, E = 2, 2048, 1024
SB = 512                # seq rows per core (= 4 heads)
N_CORES = 8


def build(reps: int = 1, phases: int = 3):
    nc = bacc.Bacc(None, target_bir_lowering=False)

    qT = nc.dram_tensor("qT", [E, SB], F16, kind="ExternalInput")
    kT = nc.dram_tensor("kT", [E, SB], F16, kind="ExternalInput")
    vT = nc.dram_tensor("vT", [E, SB], F16, kind="ExternalInput")
    wqT = nc.dram_tensor("wqT", [E, E], F16, kind="ExternalInput")
    wkT = nc.dram_tensor("wkT", [E, E], F16, kind="ExternalInput")
    wvT = nc.dram_tensor("wvT", [E, E], F16, kind="ExternalInput")
    wpT = nc.dram_tensor("wpT", [E, E], F16, kind="ExternalInput")
    bq = nc.dram_tensor("bq", [1, E], F16, kind="ExternalInput")
    bk = nc.dram_tensor("bk", [1, E], F16, kind="ExternalInput")
    bv = nc.dram_tensor("bv", [1, E], F16, kind="ExternalInput")
    bp = nc.dram_tensor("bp", [1, E], F16, kind="ExternalInput")
    y = nc.dram_tensor("y", [SB, E], F32, kind="ExternalOutput")

    with tile.TileContext(nc) as tc:
        with (
            tc.tile_pool(name="consts", bufs=1) as consts,
            tc.tile_pool(name="wpool", bufs=1) as wpool,
            tc.tile_pool(name="proj", bufs=2) as proj,
            tc.tile_pool(name="attn", bufs=1) as attn,
            tc.tile_pool(name="ptile", bufs=3) as ptile,
            tc.tile_pool(name="ypool", bufs=2) as ypool,
            tc.tile_pool(name="ps", bufs=3, space="PSUM") as ps,
            tc.tile_pool(name="dram", bufs=1, space="DRAM") as dram,
        ):
            # ---- constants -------------------------------------------------
            ones128 = consts.tile([1, 128], F16)
            nc.vector.memset(ones128, 1.0)
            bias_sb = {}
            for nm, t in (("q", bq), ("k", bk), ("v", bv), ("p", bp)):
                b_t = consts.tile([1, E], F16, name=f"bias_{nm}")
                nc.sync.dma_start(out=b_t, in_=t[:, :])
                bias_sb[nm] = b_t

            # ---- weight/activation tiles; q/k loaded now, v/p deferred -----
            w_sb, x_sb, dram_in = {}, {}, {}
            for nm, wt, xt in (("q", wqT, qT), ("k", wkT, kT), ("v", wvT, vT)):
                w_t = wpool.tile([128, 8, E], F16, name=f"w_{nm}")
                x_t = wpool.tile([128, 8, SB], F16, name=f"x_{nm}")
                dram_in[nm] = (wt, xt)
                w_sb[nm], x_sb[nm] = w_t, x_t
            w_p = wpool.tile([128, 8, E], F16, name="w_p")
            w_sb["p"] = w_p
            dram_in["p"] = (wpT, None)

            def load_inputs(nm, eng=None):
                eng = eng or nc.sync
                wt, xt = dram_in[nm]
                wre = wt.ap().rearrange("(t p) f -> p t f", p=128)
                if xt is not None:
                    xre = xt.ap().rearrange("(t p) s -> p t s", p=128)
                    eng.dma_start(out=x_sb[nm], in_=xre)
                for t2 in range(2):
                    eng.dma_start(out=w_sb[nm][:, 4 * t2:4 * t2 + 4],
                                  in_=wre[:, 4 * t2:4 * t2 + 4])

            load_inputs("q")
            load_inputs("k")

            # ---- DRAM scratch ---------------------------------------------
            qkp = [dram.tile([2 * S, 128], F16, name=f"qkp{i}")
                   for i in range(2)]
            vh_d = dram.tile([SB, E], F16)

            for rep in range(reps):
                _body(nc, tc, ps, proj, attn, ptile, ypool,
                      ones128, bias_sb, w_sb, x_sb, qkp, vh_d, y,
                      rep, phases, load_inputs if rep == 0 else None)
    nc.finalize()
    return nc


def _body(nc, tc, ps, proj, attn, ptile, ypool, ones128,
          bias_sb, w_sb, x_sb, qkp, vh_d, y, rep, phases=3,
          load_inputs=None):
    xT2 = attn.tile([128, 8, SB], F16, tag="xT2", name=f"xT2_{rep}")
    if phases < 2:
        nc.vector.memset(xT2[:, 0, 0:1], 0.0)
    _xh_cache = {}

    def project_unit(st, nm, ch):
        # one psum-group of the projection for (seq-tile st, proj nm, chunk ch)
        xh = _xh_cache.get((st, nm))
        if xh is None:
            xh = proj.tile([128, E], F16, tag="xh", name=f"xh_{nm}{st}_{rep}")
            _xh_cache[(st, nm)] = xh
        pp = ps.tile([128, 512], F32, tag="P1", bufs=2, name=f"pp{rep}")
        nc.tensor.matmul(pp, ones128[0:1, :],
                         bias_sb[nm][0:1, bass.ts(ch, 512)],
                         start=True, stop=False)
        for t in range(8):
            nc.tensor.matmul(
                pp,
                x_sb[nm][:, t, bass.ts(st, 128)],
                w_sb[nm][:, t, bass.ts(ch, 512)],
                start=False, stop=(t == 7))
        nc.vector.tensor_copy(xh[:, bass.ts(ch, 512)], pp)
        if ch == 1:
            if nm == "v":
                nc.sync.dma_start(out=vh_d[bass.ts(st, 128), :], in_=xh)
            else:
                tgt = qkp[st // 2]
                base = (0 if nm == "q" else S * 128) + 64 * (st % 2)
                out_ap = bass.AP(
                    tgt.tensor, tgt.offset + base,
                    [[2048, 128], [128, 16], [1, 64]])
                nc.sync.dma_start(
                    out=out_ap, in_=xh.rearrange("r (c d) -> r c d", d=64))

    def project(st):
        for nm in ("q", "k", "v"):
            for ch in range(2):
                project_unit(st, nm, ch)

    def attend_load(pair):
        QKT = ptile.tile([128, 2 * S], F16, tag="QKT", bufs=2,
                         name=f"QKT{pair}_{rep}")
        nc.scalar.dma_start(out=QKT, in_=qkp[pair][:, :], transpose=True)
        return QKT[:, 0:S], QKT[:, S:2 * S]

    def attend(pair, loaded, fillers=(), tail_fill=((), ())):
        QT, KT = loaded
        fillers = list(fillers)
        vps = []
        for half in range(2):
            h = 2 * pair + half
            vp = ptile.tile([128, 16, 65], F16, tag="vp", bufs=4,
                            name=f"vp{h}_{rep}")
            v_src = bass.AP(vh_d.tensor, vh_d.offset + 128 * h * E,
                            [[64, 128], [8192, 16], [1, 64]])
            nc.sync.dma_start(out=vp[:, :, 0:64], in_=v_src)
            nc.vector.memset(vp[:, :, 64:65], 1.0)
            vps.append(vp)

        # per-head SBUF accumulators for out^T (+rowsum row 64)
        osb = [ptile.tile([65, 2048], F32, tag="osb", bufs=3,
                          name=f"osb{2 * pair + half}_{rep}")
               for half in range(2)]

        LAG = 2   # defer V-matmuls 2 groups behind S^T/exp (pt bufs cover it)
        pending = []

        def emit_vmms(ent):
            qc_, js_, pts_, psO_ = ent
            jmax_ = 4 * qc_ + 3
            for half in range(2):
                pt = pts_[half]
                for jj, j in enumerate(js_):
                    o = j - 4 * qc_
                    lo = 0 if o < 0 else 128 * o
                    nc.tensor.matmul(
                        psO_[half][:, lo:],
                        vps[half][:, j, :],
                        pt[:, 512 * jj + lo:512 * jj + 512],
                        start=(j == 0), stop=(j == jmax_))

        for qc in (1, 0, 2, 3):
            jmax = 4 * qc + 3
            psO = [ps.tile([65, 512], F32, tag="O", bufs=2,
                           name=f"psO{2 * pair + half}_{qc}_{rep}")
                   for half in range(2)]
            for j0 in range(0, jmax + 1, 2):
                js = [j for j in (j0, j0 + 1) if j <= jmax]
                lo0 = max(0, 128 * (js[0] - 4 * qc))
                pts = []
                for half in range(2):
                    psS = ps.tile([128, 1024], F32, tag="S", bufs=2,
                                  name=f"psS{half}_{qc}_{j0}_{rep}")
                    pt = ptile.tile([128, 1024], F16, tag="P", bufs=4,
                                    name=f"pt{half}_{qc}_{j0}_{rep}")
                    r0, r1 = 64 * half, 64 * half + 64
                    for jj, j in enumerate(js):
                        o = j - 4 * qc
                        lo = 0 if o < 0 else 128 * o
                        nc.tensor.matmul(
                            psS[:, 512 * jj + lo:512 * jj + 512],
                            KT[r0:r1, bass.ts(j, 128)],
                            QT[r0:r1, 512 * qc + lo:512 * qc + 512],
                            start=True, stop=True)
                    # one exp per group; stale lead-in cols are never read
                    nc.scalar.activation(pt[:, lo0:], psS[:, lo0:], EXP)
                    pts.append(pt)
                    for jj, j in enumerate(js):
                        o = j - 4 * qc
                        if o >= 0:
                            sl = pts[half][:, 512 * jj + 128 * o:
                                           512 * jj + 128 * o + 128]
                            nc.gpsimd.affine_select(
                                out=sl, in_=sl,
                                pattern=[[1, 128]],
                                compare_op=mybir.AluOpType.is_ge,
                                fill=0.0, base=0, channel_multiplier=-1)
                if fillers:
                    fillers.pop(0)()   # independent PE work while exp runs
                pending.append((qc, js, pts, psO))
                if len(pending) > LAG:
                    emit_vmms(pending.pop(0))
                if fillers:
                    fillers.pop(0)()
            # drain this qc's V-matmuls before evacuating its psO
            while pending:
                emit_vmms(pending.pop(0))
            for half in range(2):
                nc.vector.tensor_copy(osb[half][:, bass.ts(qc, 512)],
                                      psO[half])

            if qc in (0, 3):
                # normalize the finished s' segment [1024*seg, 1024*(seg+1))
                seg = 0 if qc == 0 else 1
                base = 1024 * seg
                for half in range(2):
                    h = 2 * pair + half
                    recip = ptile.tile([1, 1024], F32, tag="recip", bufs=4,
                                       name=f"recip{h}{seg}_{rep}")
                    nc.vector.reciprocal(recip,
                                         osb[half][64:65, base:base + 1024])
                    bsb = ptile.tile([64, 1024], F32, tag="bsb", bufs=4,
                                     name=f"bsb{h}{seg}_{rep}")
                    nc.gpsimd.partition_broadcast(bsb, recip)
                    o_re = osb[half][0:64, base:base + 1024].rearrange(
                        "p (r c) -> p c r", c=16)
                    b_re = bsb.rearrange("p (r c) -> p c r", c=16)
                    for t in range(8):
                        for h2 in range(2):
                            c = 2 * t + h2
                            nc.vector.tensor_tensor(
                                xT2[64 * h2:64 * h2 + 64, t,
                                    128 * h + 64 * seg:
                                    128 * h + 64 * seg + 64],
                                b_re[:, c, :], o_re[:, c, :],
                                op=mybir.AluOpType.mult)
                    if seg == 1:
                        for f in tail_fill[half]:
                            f()

        for f in fillers:
            f()

    def final_unit(st, ch):
            py = ps.tile([128, 512], F32, tag="P1", bufs=2,
                         name=f"py{st}{ch}_{rep}")
            # xT2-dependent matmul first so the psum slot isn't grabbed early
            for t in range(8):
                nc.tensor.matmul(py,
                                 xT2[:, t, bass.ts(st, 128)],
                                 w_sb["p"][:, t, bass.ts(ch, 512)],
                                 start=(t == 0), stop=False)
            nc.tensor.matmul(py, ones128[0:1, :],
                             bias_sb["p"][0:1, bass.ts(ch, 512)],
                             start=False, stop=True)
            ysb = ypool.tile([128, 512], F32, tag="y",
                             name=f"ysb{st}{ch}_{rep}")
            nc.scalar.copy(ysb, py)
            nc.sync.dma_start(out=y[bass.ts(st, 128), bass.ts(ch, 512)],
                                in_=ysb)

    def final(st):
        for ch in range(2):
            final_unit(st, ch)

    # pipeline: proj st0/st1 dense; pair-0 attention with proj st2/st3 as
    # PE fillers; pair-1 attention with final st0/st1 as fillers; tail.
    _xh_cache.clear()
    if phases < 2:
        if load_inputs is not None:
            load_inputs("v")
        for st in range(4):
            project(st)
        return
    # q/k projections of tiles 0/1 first so pair-0 transposes start early
    for st, nm in ((0, "q"), (1, "q"), (0, "k"), (1, "k")):
        for ch in range(2):
            project_unit(st, nm, ch)
    loaded0 = attend_load(0)
    if load_inputs is not None:
        load_inputs("v", nc.scalar)
    for st in (0, 1):
        for ch in range(2):
            project_unit(st, "v", ch)
    loaded1_box = {}
    fill0 = [
        (lambda st=st, nm=nm, ch=ch: project_unit(st, nm, ch))
        for nm in ("q", "k") for st in (2, 3) for ch in range(2)
    ] + [
        lambda: loaded1_box.update(v=attend_load(1))
    ] + [
        (lambda st=st, ch=ch: project_unit(st, "v", ch))
        for st in (2, 3) for ch in range(2)
    ]
    attend(0, loaded0, fill0)
    if load_inputs is not None:
        load_inputs("p")
    loaded1 = loaded1_box["v"]
    if phases >= 3:
        fill1 = [
            (lambda st=st, ch=ch: final_unit(st, ch))
            for st in (0, 1) for ch in range(2)
        ]
        tails = ([(lambda ch=ch: final_unit(2, ch)) for ch in range(2)],
                 [(lambda ch=ch: final_unit(3, ch)) for ch in range(2)])
    else:
        fill1, tails = [], ((), ())
    attend(1, loaded1, fill1, tail_fill=tails)


# ---------------------------------------------------------------------------
# host side
# ---------------------------------------------------------------------------

_CACHE = {}


def _prep_inputs(q, k, v, Wq, bq, Wk, bk, Wv, bv, Wp, bp):
    scale = 1.0 / np.sqrt(64.0)
    wq_T = np.ascontiguousarray(np.asarray(Wq, np.float32).T).astype(np.float16)
    wk_T = np.ascontiguousarray(np.asarray(Wk, np.float32).T).astype(np.float16)
    wv_T = np.ascontiguousarray(np.asarray(Wv, np.float32).T).astype(np.float16)
    wp_T = np.ascontiguousarray(np.asarray(Wp, np.float32).T).astype(np.float16)
    shared = {
        "wqT": wq_T, "wkT": wk_T, "wvT": wv_T, "wpT": wp_T,
        "bq": (np.asarray(bq, np.float32) * scale).astype(np.float16)[None, :],
        "bk": np.asarray(bk, np.float16)[None, :],
        "bv": np.asarray(bv, np.float16)[None, :],
        "bp": np.asarray(bp, np.float16)[None, :],
    }
    in_maps = []
    for c in range(N_CORES):
        b, g = divmod(c, 4)
        rows = slice(SB * g, SB * (g + 1))
        m = dict(shared)
        m["qT"] = np.ascontiguousarray(
            np.asarray(q[b, rows], np.float32).T * scale).astype(np.float16)
        m["kT"] = np.ascontiguousarray(
            np.asarray(k[b, rows], np.float32).T).astype(np.float16)
        m["vT"] = np.ascontiguousarray(
            np.asarray(v[b, rows], np.float32).T).astype(np.float16)
        in_maps.append(m)
    return in_maps


def kernel(q, k, v, Wq, bq, Wk, bk, Wv, bv, Wp, bp):
    if "nc" not in _CACHE:
        _CACHE["nc"] = build()
    nc = _CACHE["nc"]
    in_maps = _prep_inputs(q, k, v, Wq, bq, Wk, bk, Wv, bv, Wp, bp)
    res = run_bass_kernel_spmd(nc, in_maps, core_ids=list(range(N_CORES)))
    out = np.empty((B, S, E), np.float32)
    for c in range(N_CORES):
        b, g = divmod(c, 4)
        out[b, SB * g:SB * (g + 1), :] = res.results[c]["y"]
    return out


if __name__ == "__main__":
    rng = np.random.default_rng(0)
    s = 1.0 / np.sqrt(E)
    ins = {
        "q": rng.standard_normal((B, S, E), dtype=np.float32),
        "k": rng.standard_normal((B, S, E), dtype=np.float32),
        "v": rng.standard_normal((B, S, E), dtype=np.float32),
        "Wq": rng.standard_normal((E, E), dtype=np.float32) * s,
        "bq": rng.standard_normal(E).astype(np.float32) * s,
        "Wk": rng.standard_normal((E, E), dtype=np.float32) * s,
        "bk": rng.standard_normal(E).astype(np.float32) * s,
        "Wv": rng.standard_normal((E, E), dtype=np.float32) * s,
        "bv": rng.standard_normal(E).astype(np.float32) * s,
        "Wp": rng.standard_normal((E, E), dtype=np.float32) * s,
        "bp": rng.standard_normal(E).astype(np.float32) * s,
    }
    out = kernel(**ins)
    print("kernel ran, out shape", out.shape, "mean", float(np.abs(out).mean()))



# revision 42
# speedup vs baseline: 1.0763x; 1.0763x over previous
"""Trainium2 Bass kernel for nn_MultiHeadAttention_66202625900642.

Reference semantics (B=2, S=2048, E=1024, H=16 heads, D=64):
    qh = q @ Wq.T + bq   (same k, v)
    head split is a PLAIN RESHAPE (B, S, E) -> (B, H, S, D):
      head h of batch b = rows [128h, 128h+128) of qh[b] reinterpreted
      row-major as a (2048, 64) matrix (scrambled seq index s' = 16r + c).
    causal softmax over s', out @ Wp.T + bp.

Because the head split partitions the *sequence* rows, sharding each batch
into 4 row-blocks of 512 (= 4 heads) is fully local: 8 cores = 2 batches x 4
quarters, zero collectives. Weights are replicated (fp16).

Per-core pipeline (all matmuls fp16, fp32 PSUM accumulation):
  1. projections -> qh/kh/vh fp16 (weights streamed per e-tile)
  2. DRAM round-trip: qh/kh into per-pair [2048, 128] files (2 heads wide),
     vh natural; DMA-transpose pair files back as [128, 2048] = two heads'
     Q_hT/K_hT stacked; vh re-read as [128, 65] V' tiles (ones column -> row
     sums ride along the P^T @ V' matmul).
  3. attention per head pair, both heads' S^T blocks issued to disjoint PE
     row groups (K=64 each -> concurrent on the 128x128 array): one exp per
     psum group on ACT, causal triangles via gpsimd affine_select,
     P^T @ V' accumulates out^T[d, s'] + rowsum into per-chunk PSUM,
     evacuated to SBUF.
  4. normalization: reciprocal of rowsum, PE-broadcast (f32r K=1 matmul),
     fused into the stride-16 rearrange to final-projection layout.
  5. final projection -> y fp32.
"""

import numpy as np

import concourse.bass as bass
import concourse.mybir as mybir
import concourse.tile as tile
from concourse import bacc
from concourse.bass_utils import run_bass_kernel_spmd

F16 = mybir.dt.float16
F32 = mybir.dt.float32
F32R = mybir.dt.float32r
EXP = mybir.ActivationFunctionType.Exp

B, S, E = 2, 2048, 1024
SB = 512                # seq rows per core (= 4 heads)
N_CORES = 8


def build(reps: int = 1, phases: int = 3):
    nc = bacc.Bacc(None, target_bir_lowering=False)

    qT = nc.dram_tensor("qT", [E, SB], F16, kind="ExternalInput")
    kT = nc.dram_tensor("kT", [E, SB], F16, kind="ExternalInput")
    vT = nc.dram_tensor("vT", [E, SB], F16, kind="ExternalInput")
    wqT = nc.dram_tensor("wqT", [E, E], F16, kind="ExternalInput")
    wkT = nc.dram_tensor("wkT", [E, E], F16, kind="ExternalInput")
    wvT = nc.dram_tensor("wvT", [E, E], F16, kind="ExternalInput")
    wpT = nc.dram_tensor("wpT", [E, E], F16, kind="ExternalInput")
    bq = nc.dram_tensor("bq", [1, E], F16, kind="ExternalInput")
    bk = nc.dram_tensor("bk", [1, E], F16, kind="ExternalInput")
    bv = nc.dram_tensor("bv", [1, E], F16, kind="ExternalInput")
    bp = nc.dram_tensor("bp", [1, E], F16, kind="ExternalInput")
    y = nc.dram_tensor("y", [SB, E], F16, kind="ExternalOutput")

    with tile.TileContext(nc) as tc:
        with (
            tc.tile_pool(name="consts", bufs=1) as consts,
            tc.tile_pool(name="wpool", bufs=1) as wpool,
            tc.tile_pool(name="proj", bufs=2) as proj,
            tc.tile_pool(name="attn", bufs=1) as attn,
            tc.tile_pool(name="ptile", bufs=3) as ptile,
            tc.tile_pool(name="ypool", bufs=2) as ypool,
            tc.tile_pool(name="ps", bufs=3, space="PSUM") as ps,
            tc.tile_pool(name="dram", bufs=1, space="DRAM") as dram,
        ):
            # ---- constants -------------------------------------------------
            ones128 = consts.tile([1, 128], F16)
            nc.vector.memset(ones128, 1.0)
            # identity for PE tile transposes
            id128 = consts.tile([128, 128], F16)
            nc.vector.memset(id128, 1.0)
            nc.gpsimd.affine_select(
                out=id128, in_=id128, pattern=[[1, 128]],
                compare_op=mybir.AluOpType.is_equal,
                fill=0.0, base=0, channel_multiplier=-1)
            bias_sb = {}
            for nm, t in (("q", bq), ("k", bk), ("v", bv), ("p", bp)):
                b_t = consts.tile([1, E], F16, name=f"bias_{nm}")
                nc.sync.dma_start(out=b_t, in_=t[:, :])
                # bias adds ride the psum->sbuf evacuation on DVE instead of
                # burning PE columns: broadcast each to all 128 partitions.
                b_bc = consts.tile([128, E], F16, name=f"biasbc_{nm}")
                nc.gpsimd.partition_broadcast(b_bc, b_t)
                bias_sb[nm] = b_bc

            # ---- weight/activation tiles; q/k loaded now, v/p deferred -----
            w_sb, x_sb, dram_in = {}, {}, {}
            for nm, wt, xt in (("q", wqT, qT), ("k", wkT, kT), ("v", wvT, vT)):
                w_t = wpool.tile([128, 8, E], F16, name=f"w_{nm}")
                x_t = wpool.tile([128, 8, SB], F16, name=f"x_{nm}")
                dram_in[nm] = (wt, xt)
                w_sb[nm], x_sb[nm] = w_t, x_t
            w_p = wpool.tile([128, 8, E], F16, name="w_p")
            w_sb["p"] = w_p
            dram_in["p"] = (wpT, None)

            def load_inputs(nm, eng=None):
                # fine-grained, t-interleaved chunks so the first projection
                # matmul starts after ~1MB instead of ~3MB (DMA device is
                # serial in the cost model); all input streams ride the ACT
                # hwdge queue, which carries no dependent (waiting) DMAs.
                eng = eng or nc.scalar
                wt, xt = dram_in[nm]
                wre = wt.ap().rearrange("(t p) f -> p t f", p=128)
                if xt is not None:
                    xre = xt.ap().rearrange("(t p) s -> p t s", p=128)
                    for t4 in range(2):
                        eng.dma_start(out=x_sb[nm][:, 4 * t4:4 * t4 + 4],
                                      in_=xre[:, 4 * t4:4 * t4 + 4])
                        for t in range(4 * t4, 4 * t4 + 4):
                            eng.dma_start(out=w_sb[nm][:, t:t + 1],
                                          in_=wre[:, t:t + 1])
                else:
                    for t2 in range(4):
                        eng.dma_start(out=w_sb[nm][:, 2 * t2:2 * t2 + 2],
                                      in_=wre[:, 2 * t2:2 * t2 + 2])

            load_inputs("q")
            load_inputs("k")

            # ---- DRAM scratch ---------------------------------------------
            vh_d = dram.tile([SB, E], F16)

            for rep in range(reps):
                _body(nc, tc, ps, proj, attn, ptile, ypool,
                      ones128, id128, bias_sb, w_sb, x_sb, vh_d, y,
                      rep, phases, load_inputs if rep == 0 else None)
    nc.finalize()
    return nc


def _body(nc, tc, ps, proj, attn, ptile, ypool, ones128, id128,
          bias_sb, w_sb, x_sb, vh_d, y, rep, phases=3,
          load_inputs=None):
    xT2 = attn.tile([128, 8, SB], F16, tag="xT2", name=f"xT2_{rep}")
    if phases < 2:
        nc.vector.memset(xT2[:, 0, 0:1], 0.0)
    _xh_cache = {}

    def project_unit(st, nm, ch, ptag="P1"):
        # one psum-group of the projection for (seq-tile st, proj nm, chunk ch)
        xh = _xh_cache.get((st, nm))
        if xh is None:
            xh = proj.tile([128, E], F16, tag="xh", name=f"xh_{nm}{st}_{rep}")
            _xh_cache[(st, nm)] = xh
        # initial-phase units borrow the (then idle) psS slots so the filler
        # tag P1 can stay at one buffer (PSUM is fully subscribed)
        pp = ps.tile([128, 512] if ptag == "P1" else [128, 1024], F32,
                     tag=ptag, bufs=1 if ptag == "P1" else 2,
                     name=f"pp{rep}")[:, 0:512]
        for t in range(8):
            nc.tensor.matmul(
                pp,
                x_sb[nm][:, t, bass.ts(st, 128)],
                w_sb[nm][:, t, bass.ts(ch, 512)],
                start=(t == 0), stop=(t == 7))
        nc.vector.tensor_tensor(xh[:, bass.ts(ch, 512)], pp,
                                bias_sb[nm][:, bass.ts(ch, 512)],
                                op=mybir.AluOpType.add)
        if ch == 1:
            if nm == "v":
                nc.sync.dma_start(out=vh_d[bass.ts(st, 128), :], in_=xh)
            else:
                # Q^T/K^T built in SBUF via PE transposes + strided DVE
                # copies: no DRAM round-trip, no DMA-transpose, no cross-queue
                # semaphore coupling. xh cols (c,d) -> QKT[64h2+d, 16p+c].
                qkt = _qkt_of(st // 2)
                half = st % 2
                off = 0 if nm == "q" else S
                tgt = qkt[64 * half:64 * half + 64,
                          off:off + S].rearrange("p (q c) -> p c q", c=16)
                for b4 in range(4):
                    psT = ps.tile([64, 4, 128], F16, tag="T", bufs=1,
                                  name=f"psT{nm}{st}{b4}_{rep}")
                    for ci in range(4):
                        c = 4 * b4 + ci
                        # one psum group per bank: first start zeroes the
                        # whole 2KB zero region, disjoint slices accumulate
                        nc.tensor.matmul(psT[:, ci],
                                         xh[:, 64 * c:64 * c + 64], id128,
                                         is_transpose=True,
                                         start=(ci == 0), stop=(ci == 3))
                    nc.vector.tensor_copy(tgt[:, 4 * b4:4 * b4 + 4], psT)

    def project(st):
        for nm in ("q", "k", "v"):
            for ch in range(2):
                project_unit(st, nm, ch, ptag="S")

    _qkt_cache = {}

    def _qkt_of(pair):
        qkt = _qkt_cache.get(pair)
        if qkt is None:
            qkt = ptile.tile([128, 2 * S], F16, tag="QKT", bufs=2,
                             name=f"QKT{pair}_{rep}")
            _qkt_cache[pair] = qkt
        return qkt

    def attend_load(pair):
        QKT = _qkt_of(pair)
        return QKT[:, 0:S], QKT[:, S:2 * S]

    def attend(pair, loaded, fillers=(), tail_fill=((), ()),
               vp_lazy=False, vp_box=None):
        QT, KT = loaded
        fillers = list(fillers)
        vps = [None, None]

        def load_vp(half):
            h = 2 * pair + half
            vp = ptile.tile([128, 16, 65], F16, tag="vp", bufs=4,
                            name=f"vp{h}_{rep}")
            v_src = bass.AP(vh_d.tensor, vh_d.offset + 128 * h * E,
                            [[64, 128], [8192, 16], [1, 64]])
            with tc.high_priority():
                nc.sync.dma_start(out=vp[:, :, 0:64], in_=v_src)
            nc.vector.memset(vp[:, :, 64:65], 1.0)
            vps[half] = vp

        if vp_box is not None:
            vp_box["f"] = load_vp
        if not vp_lazy:
            load_vp(0)
            load_vp(1)

        LAG = 3   # defer V-matmuls 3 groups behind S^T/exp (pt bufs cover it)
        pending = []
        deferred = []   # finish_b closures, run 1-2 steps after their qc ends

        def emit_vmms(ent):
            # P^T@V with tall [128q, 65] outputs: per (j, qsub) block the PE
            # charge is 65 cols instead of 512 (M/K are free in the model).
            qc_, js_, pts_, po_ = ent
            for half in range(2):
                if vps[half] is None:
                    load_vp(half)
                pt = pts_[half]
                for jj, j in enumerate(js_):
                    for qs in range(4):
                        if 4 * qc_ + qs < j:
                            continue
                        # single psum group per (half, qc) bank: start only
                        # on the very first block, stop on the very last
                        nc.tensor.matmul(
                            po_[half][:, qs, :],
                            pt[:, 512 * jj + 128 * qs:
                               512 * jj + 128 * qs + 128],
                            vps[half][:, j, :],
                            start=(j == 0 and qs == 0),
                            stop=(j == 4 * qc_ + 3 and qs == 3))

        def finish_a(half, qc, po):
            # DVE part: rowsums -> reciprocals -> normalized [q, d] tiles.
            # Frees the po psum buffer; PE transposes come later (finish_b)
            # so the in-order PE stream never waits on this chain.
            h = 2 * pair + half
            sums = ptile.tile([128, 4, 1], F32, tag="sums", bufs=2,
                              name=f"sums{h}{qc}_{rep}")
            nc.vector.tensor_copy(sums, po[:, :, 64:65])
            rec = ptile.tile([128, 4, 1], F32, tag="rec", bufs=2,
                             name=f"rec{h}{qc}_{rep}")
            nc.vector.reciprocal(rec, sums)
            pn = ptile.tile([128, 4, 64], F16, tag="pn", bufs=2,
                            name=f"pn{h}{qc}_{rep}")
            for qs in range(4):
                nc.vector.tensor_scalar_mul(pn[:, qs], po[:, qs, 0:64],
                                            rec[:, qs])
            return pn

        def finish_b(half, qc, pn):
            # PE transposes [128q,64d] -> [64d,128q], then strided copies
            # into the final-projection layout xT2.
            h = 2 * pair + half
            po2 = ps.tile([64, 512], F16, tag="T", bufs=1,
                          name=f"po2{h}{qc}_{rep}")
            for qs in range(4):
                nc.tensor.matmul(po2[:, bass.ts(qs, 128)], pn[:, qs],
                                 id128, is_transpose=True,
                                 start=(qs == 0), stop=(qs == 3))
            # po2 free index = 16r + 2c2 + h2  ->  xT2[64h2+d, c2, 32qc + r]
            p_re = po2.rearrange("p (r c2 h2) -> p h2 c2 r", c2=8, h2=2)
            for h2 in range(2):
                nc.vector.tensor_copy(
                    xT2[64 * h2:64 * h2 + 64, :,
                        128 * h + 32 * qc:128 * h + 32 * qc + 32],
                    p_re[:, h2])
            if qc == 3:
                for f in tail_fill[half]:
                    f()

        def pop_pending():
            ent = pending.pop(0)
            emit_vmms(ent)
            qc_, js_ = ent[0], ent[1]
            if js_[-1] == 4 * qc_ + 3:   # last group of its qc
                for half in range(2):
                    pn = finish_a(half, qc_, ent[3][half])
                    deferred.append(
                        lambda half=half, qc_=qc_, pn=pn:
                        finish_b(half, qc_, pn))

        po_of = {}
        for qc in (1, 0, 2, 3):
            jmax = 4 * qc + 3
            po_of[qc] = [ps.tile([128, 4, 65], F32, tag="O", bufs=2,
                                 name=f"po{2 * pair + half}_{qc}_{rep}")
                         for half in range(2)]
            for j0 in range(0, jmax + 1, 2):
                js = [j for j in (j0, j0 + 1) if j <= jmax]
                pts = []
                for half in range(2):
                    psS = ps.tile([128, 1024], F32, tag="S", bufs=2,
                                  name=f"psS{half}_{qc}_{j0}_{rep}")
                    pt = ptile.tile([128, 1024], F16, tag="P", bufs=4,
                                    name=f"pt{half}_{qc}_{j0}_{rep}")
                    r0, r1 = 64 * half, 64 * half + 64
                    exp_runs = []    # (lo, hi) spans to exp, exact width
                    for jj, j in enumerate(js):
                        o = j - 4 * qc
                        lo = 0 if o < 0 else 128 * o
                        nc.tensor.matmul(
                            psS[:, 512 * jj + lo:512 * jj + 512],
                            KT[r0:r1, bass.ts(j, 128)],
                            QT[r0:r1, 512 * qc + lo:512 * qc + 512],
                            start=True, stop=True)
                        lo_, hi_ = 512 * jj + lo, 512 * jj + 512
                        if exp_runs and exp_runs[-1][1] == lo_:
                            exp_runs[-1] = (exp_runs[-1][0], hi_)
                        else:
                            exp_runs.append((lo_, hi_))
                    for lo_, hi_ in exp_runs:
                        nc.scalar.activation(pt[:, lo_:hi_],
                                             psS[:, lo_:hi_], EXP)
                    pts.append(pt)
                    for jj, j in enumerate(js):
                        o = j - 4 * qc
                        if o >= 0:
                            sl = pts[half][:, 512 * jj + 128 * o:
                                           512 * jj + 128 * o + 128]
                            nc.gpsimd.affine_select(
                                out=sl, in_=sl,
                                pattern=[[1, 128]],
                                compare_op=mybir.AluOpType.is_ge,
                                fill=0.0, base=0, channel_multiplier=-1)
                if fillers:
                    fillers.pop(0)()   # independent PE work while exp runs
                pending.append((qc, js, pts, po_of[qc]))
                if deferred:
                    deferred.pop(0)()
                if len(pending) > LAG:
                    pop_pending()
                if fillers:
                    fillers.pop(0)()
        # drain
        while pending:
            pop_pending()
        while deferred:
            deferred.pop(0)()
        for f in fillers:
            f()

    def final_unit(st, ch, ptag="P1"):
            # tail units borrow the psS slots (attention is over by then)
            py = ps.tile([128, 512] if ptag == "P1" else [128, 1024], F32,
                         tag=ptag, bufs=1 if ptag == "P1" else 2,
                         name=f"py{st}{ch}_{rep}")[:, 0:512]
            for t in range(8):
                nc.tensor.matmul(py,
                                 xT2[:, t, bass.ts(st, 128)],
                                 w_sb["p"][:, t, bass.ts(ch, 512)],
                                 start=(t == 0), stop=(t == 7))
            ysb = ypool.tile([128, 512], F16, tag="y",
                             name=f"ysb{st}{ch}_{rep}")
            nc.vector.tensor_tensor(ysb, py,
                                    bias_sb["p"][:, bass.ts(ch, 512)],
                                    op=mybir.AluOpType.add)
            nc.sync.dma_start(out=y[bass.ts(st, 128), bass.ts(ch, 512)],
                              in_=ysb)

    def final(st):
        for ch in range(2):
            final_unit(st, ch)

    # pipeline: proj st0/st1 dense; pair-0 attention with proj st2/st3 as
    # PE fillers; pair-1 attention with final st0/st1 as fillers; tail.
    _xh_cache.clear()
    if phases < 2:
        if load_inputs is not None:
            load_inputs("v")
        for st in range(4):
            project(st)
        return
    def _load_p():
        # demote the final-projection weight stream: it must never win the
        # DMA device or a sem slot from attention-critical transfers
        with tc.high_priority(offset=-(10 ** 6)):
            load_inputs("p")

    # q/k projections of tiles 0/1 first so pair-0 transposes start early;
    # q st2/3 is also pure-SBUF work by then. v projections go to fillers so
    # the in-order PE stream never parks on the v input stream.
    for st, nm in ((0, "q"), (1, "q"), (0, "k"), (1, "k"),
                   (2, "q"), (3, "q")):
        for ch in range(2):
            project_unit(st, nm, ch, ptag="S")
    loaded0 = attend_load(0)
    if load_inputs is not None:
        load_inputs("v")
    loaded1_box = {}
    vp0_box = {}
    fill0 = [
        (lambda st=st, ch=ch: project_unit(st, "v", ch))
        for st in (0, 1) for ch in range(2)
    ] + [
        (lambda st=st, ch=ch: project_unit(st, "k", ch))
        for st in (2, 3) for ch in range(2)
    ] + [
        lambda: loaded1_box.update(v=attend_load(1)),
        (lambda: _load_p()) if load_inputs is not None else (lambda: None),
    ] + [
        (lambda st=st, ch=ch: project_unit(st, "v", ch))
        for st in (2, 3) for ch in range(2)
    ]
    attend(0, loaded0, fill0, vp_lazy=True, vp_box=vp0_box)
    loaded1 = loaded1_box["v"]
    if phases >= 3:
        fill1 = [
            (lambda st=st, ch=ch: final_unit(st, ch))
            for st in (0, 1) for ch in range(2)
        ]
        tails = ([(lambda ch=ch: final_unit(2, ch, ptag="S"))
                  for ch in range(2)],
                 [(lambda ch=ch: final_unit(3, ch, ptag="S"))
                  for ch in range(2)])
    else:
        fill1, tails = [], ((), ())
    attend(1, loaded1, fill1, tail_fill=tails)


# ---------------------------------------------------------------------------
# host side
# ---------------------------------------------------------------------------

_CACHE = {}


def _prep_inputs(q, k, v, Wq, bq, Wk, bk, Wv, bv, Wp, bp):
    scale = 1.0 / np.sqrt(64.0)
    wq_T = np.ascontiguousarray(np.asarray(Wq, np.float32).T).astype(np.float16)
    wk_T = np.ascontiguousarray(np.asarray(Wk, np.float32).T).astype(np.float16)
    wv_T = np.ascontiguousarray(np.asarray(Wv, np.float32).T).astype(np.float16)
    wp_T = np.ascontiguousarray(np.asarray(Wp, np.float32).T).astype(np.float16)
    shared = {
        "wqT": wq_T, "wkT": wk_T, "wvT": wv_T, "wpT": wp_T,
        "bq": (np.asarray(bq, np.float32) * scale).astype(np.float16)[None, :],
        "bk": np.asarray(bk, np.float16)[None, :],
        "bv": np.asarray(bv, np.float16)[None, :],
        "bp": np.asarray(bp, np.float16)[None, :],
    }
    in_maps = []
    for c in range(N_CORES):
        b, g = divmod(c, 4)
        rows = slice(SB * g, SB * (g + 1))
        m = dict(shared)
        m["qT"] = np.ascontiguousarray(
            np.asarray(q[b, rows], np.float32).T * scale).astype(np.float16)
        m["kT"] = np.ascontiguousarray(
            np.asarray(k[b, rows], np.float32).T).astype(np.float16)
        m["vT"] = np.ascontiguousarray(
            np.asarray(v[b, rows], np.float32).T).astype(np.float16)
        in_maps.append(m)
    return in_maps


def kernel(q, k, v, Wq, bq, Wk, bk, Wv, bv, Wp, bp):
    if "nc" not in _CACHE:
        _CACHE["nc"] = build()
    nc = _CACHE["nc"]
    in_maps = _prep_inputs(q, k, v, Wq, bq, Wk, bk, Wv, bv, Wp, bp)
    res = run_bass_kernel_spmd(nc, in_maps, core_ids=list(range(N_CORES)))
    out = np.empty((B, S, E), np.float32)
    for c in range(N_CORES):
        b, g = divmod(c, 4)
        out[b, SB * g:SB * (g + 1), :] = res.results[c]["y"]
    return out


if __name__ == "__main__":
    rng = np.random.default_rng(0)
    s = 1.0 / np.sqrt(E)
    ins = {
        "q": rng.standard_normal((B, S, E), dtype=np.float32),
        "k": rng.standard_normal((B, S, E), dtype=np.float32),
        "v": rng.standard_normal((B, S, E), dtype=np.float32),
        "Wq": rng.standard_normal((E, E), dtype=np.float32) * s,
        "bq": rng.standard_normal(E).astype(np.float32) * s,
        "Wk": rng.standard_normal((E, E), dtype=np.float32) * s,
        "bk": rng.standard_normal(E).astype(np.float32) * s,
        "Wv": rng.standard_normal((E, E), dtype=np.float32) * s,
        "bv": rng.standard_normal(E).astype(np.float32) * s,
        "Wp": rng.standard_normal((E, E), dtype=np.float32) * s,
        "bp": rng.standard_normal(E).astype(np.float32) * s,
    }
    out = kernel(**ins)
    print("kernel ran, out shape", out.shape, "mean", float(np.abs(out).mean()))



# revision 65
# speedup vs baseline: 1.1917x; 1.1072x over previous
"""Trainium2 Bass kernel for nn_MultiHeadAttention_66202625900642.

Reference semantics (B=2, S=2048, E=1024, H=16 heads, D=64):
    qh = q @ Wq.T + bq   (same k, v)
    head split is a PLAIN RESHAPE (B, S, E) -> (B, H, S, D):
      head h of batch b = rows [128h, 128h+128) of qh[b] reinterpreted
      row-major as a (2048, 64) matrix (scrambled seq index s' = 16r + c).
    causal softmax over s', out @ Wp.T + bp.

Because the head split partitions the *sequence* rows, sharding each batch
into 4 row-blocks of 512 (= 4 heads) is fully local: 8 cores = 2 batches x 4
quarters, zero collectives. Weights are replicated (fp16).

Per-core pipeline (all matmuls fp16, fp32 PSUM accumulation):
  1. projections -> qh/kh/vh fp16 (weights streamed per e-tile)
  2. DRAM round-trip: qh/kh into per-pair [2048, 128] files (2 heads wide),
     vh natural; DMA-transpose pair files back as [128, 2048] = two heads'
     Q_hT/K_hT stacked; vh re-read as [128, 65] V' tiles (ones column -> row
     sums ride along the P^T @ V' matmul).
  3. attention per head pair, both heads' S^T blocks issued to disjoint PE
     row groups (K=64 each -> concurrent on the 128x128 array): one exp per
     psum group on ACT, causal triangles via gpsimd affine_select,
     P^T @ V' accumulates out^T[d, s'] + rowsum into per-chunk PSUM,
     evacuated to SBUF.
  4. normalization: reciprocal of rowsum, PE-broadcast (f32r K=1 matmul),
     fused into the stride-16 rearrange to final-projection layout.
  5. final projection -> y fp32.
"""

import numpy as np

import concourse.bass as bass
import concourse.mybir as mybir
import concourse.tile as tile
from concourse import bacc
from concourse.bass_utils import run_bass_kernel_spmd

F16 = mybir.dt.float16
F32 = mybir.dt.float32
F32R = mybir.dt.float32r
EXP = mybir.ActivationFunctionType.Exp

B, S, E = 2, 2048, 1024
SB = 512                # seq rows per core (= 4 heads)
N_CORES = 8


def build(reps: int = 1, phases: int = 3):
    nc = bacc.Bacc(None, target_bir_lowering=False)

    qT = nc.dram_tensor("qT", [E, SB], F16, kind="ExternalInput")
    kT = nc.dram_tensor("kT", [E, SB], F16, kind="ExternalInput")
    vT = nc.dram_tensor("vT", [E, SB], F16, kind="ExternalInput")
    wqT = nc.dram_tensor("wqT", [E, E], F16, kind="ExternalInput")
    wkT = nc.dram_tensor("wkT", [E, E], F16, kind="ExternalInput")
    wvT = nc.dram_tensor("wvT", [E, E], F16, kind="ExternalInput")
    wpT = nc.dram_tensor("wpT", [E, E], F16, kind="ExternalInput")
    bq = nc.dram_tensor("bq", [1, E], F16, kind="ExternalInput")
    bk = nc.dram_tensor("bk", [1, E], F16, kind="ExternalInput")
    bv = nc.dram_tensor("bv", [1, E], F16, kind="ExternalInput")
    bp = nc.dram_tensor("bp", [1, E], F16, kind="ExternalInput")
    y = nc.dram_tensor("y", [SB, E], F16, kind="ExternalOutput")

    with tile.TileContext(nc) as tc:
        with (
            tc.tile_pool(name="consts", bufs=1) as consts,
            tc.tile_pool(name="wpool", bufs=1) as wpool,
            tc.tile_pool(name="proj", bufs=2) as proj,
            tc.tile_pool(name="attn", bufs=1) as attn,
            tc.tile_pool(name="ptile", bufs=3) as ptile,
            tc.tile_pool(name="ypool", bufs=2) as ypool,
            tc.tile_pool(name="ps", bufs=3, space="PSUM") as ps,
            tc.tile_pool(name="dram", bufs=1, space="DRAM") as dram,
        ):
            # ---- constants -------------------------------------------------
            ones128 = consts.tile([1, 128], F16)
            nc.vector.memset(ones128, 1.0)
            # identity for PE tile transposes
            id128 = consts.tile([128, 128], F16)
            nc.vector.memset(id128, 1.0)
            nc.gpsimd.affine_select(
                out=id128, in_=id128, pattern=[[1, 128]],
                compare_op=mybir.AluOpType.is_equal,
                fill=0.0, base=0, channel_multiplier=-1)
            bias_sb = {}
            for nm, t in (("q", bq), ("k", bk), ("v", bv), ("p", bp)):
                b_t = consts.tile([1, E], F16, name=f"bias_{nm}")
                nc.sync.dma_start(out=b_t, in_=t[:, :])
                # bias adds ride the psum->sbuf evacuation on DVE instead of
                # burning PE columns: broadcast each to all 128 partitions.
                b_bc = consts.tile([128, E], F16, name=f"biasbc_{nm}")
                nc.gpsimd.partition_broadcast(b_bc, b_t)
                bias_sb[nm] = b_bc

            # ---- weight/activation tiles; q/k loaded now, v/p deferred -----
            w_sb, x_sb, dram_in = {}, {}, {}
            for nm, wt, xt in (("q", wqT, qT), ("k", wkT, kT), ("v", wvT, vT)):
                w_t = wpool.tile([128, 8, E], F16, name=f"w_{nm}")
                x_t = wpool.tile([128, 8, SB], F16, name=f"x_{nm}")
                dram_in[nm] = (wt, xt)
                w_sb[nm], x_sb[nm] = w_t, x_t
            w_p = wpool.tile([128, 8, E], F16, name="w_p")
            w_sb["p"] = w_p
            dram_in["p"] = (wpT, None)

            def load_inputs(nm, eng=None):
                # fine-grained, t-interleaved chunks so the first projection
                # matmul starts after ~1MB instead of ~3MB (DMA device is
                # serial in the cost model). SP queue: ACT's sequencer must
                # stay free for exp dispatch (no exec queue on ACT).
                eng = eng or nc.sync
                wt, xt = dram_in[nm]
                wre = wt.ap().rearrange("(t p) f -> p t f", p=128)
                if xt is not None:
                    xre = xt.ap().rearrange("(t p) s -> p t s", p=128)
                    for t4 in range(2):
                        eng.dma_start(out=x_sb[nm][:, 4 * t4:4 * t4 + 4],
                                      in_=xre[:, 4 * t4:4 * t4 + 4])
                        for t in range(4 * t4, 4 * t4 + 4):
                            eng.dma_start(out=w_sb[nm][:, t:t + 1],
                                          in_=wre[:, t:t + 1])
                else:
                    for t2 in range(4):
                        eng.dma_start(out=w_sb[nm][:, 2 * t2:2 * t2 + 2],
                                      in_=wre[:, 2 * t2:2 * t2 + 2])

            load_inputs("q")
            load_inputs("k")

            # ---- DRAM scratch ---------------------------------------------
            vh_d = dram.tile([SB, E], F16)

            for rep in range(reps):
                _body(nc, tc, ps, proj, attn, ptile, ypool,
                      ones128, id128, bias_sb, w_sb, x_sb, vh_d, y,
                      rep, phases, load_inputs if rep == 0 else None)
    nc.finalize()
    return nc


def _body(nc, tc, ps, proj, attn, ptile, ypool, ones128, id128,
          bias_sb, w_sb, x_sb, vh_d, y, rep, phases=3,
          load_inputs=None):
    xT2 = attn.tile([128, 8, SB], F16, tag="xT2", name=f"xT2_{rep}")
    if phases < 2:
        nc.vector.memset(xT2[:, 0, 0:1], 0.0)
    _xh_cache = {}

    def project_unit(st, nm, ch, ptag="P1"):
        # one psum-group of the projection for (seq-tile st, proj nm, chunk ch)
        xh = _xh_cache.get((st, nm))
        if xh is None:
            xh = proj.tile([128, E], F16, tag="xh", name=f"xh_{nm}{st}_{rep}")
            _xh_cache[(st, nm)] = xh
        # initial-phase units borrow the (then idle) psS slots so the filler
        # tag P1 can stay at one buffer (PSUM is fully subscribed)
        pp = ps.tile([128, 512] if ptag == "P1" else [128, 1024], F32,
                     tag=ptag, bufs=1 if ptag == "P1" else 2,
                     name=f"pp{rep}")[:, 0:512]
        for t in range(8):
            nc.tensor.matmul(
                pp,
                x_sb[nm][:, t, bass.ts(st, 128)],
                w_sb[nm][:, t, bass.ts(ch, 512)],
                start=(t == 0), stop=(t == 7))
        nc.vector.tensor_tensor(xh[:, bass.ts(ch, 512)], pp,
                                bias_sb[nm][:, bass.ts(ch, 512)],
                                op=mybir.AluOpType.add)
        if nm == "v":
            if ch == 1:
                nc.sync.dma_start(out=vh_d[bass.ts(st, 128), :], in_=xh)
        else:
            # Q^T/K^T built in SBUF via PE transposes + strided DVE copies:
            # no DRAM round-trip, no DMA-transpose, no cross-queue semaphore
            # coupling. xh cols (c,d) -> QKT[64h2+d, 16p+c]. Emitted per ch
            # (each ch covers half the c's) to shorten the critical chain.
            qkt = _qkt_of(st // 2)
            half = st % 2
            off = 0 if nm == "q" else S
            tgt = qkt[64 * half:64 * half + 64, off:off + S].rearrange(
                "p (q b c2 cb) -> p b cb c2 q", b=4, c2=2, cb=2)
            for b4 in (0, 1) if ch == 0 else (2, 3):
                # 128x128 transposes: two c-columns per pass (PE charge
                # is output cols only), alternating psum tags for depth
                ptag2, pbufs = ("T", 1) if b4 % 2 == 0 else ("O", 2)
                psT = ps.tile([128, 2, 128], F16, tag=ptag2, bufs=pbufs,
                              name=f"psT{nm}{st}{b4}_{rep}")
                for ci in range(2):
                    c0 = 4 * b4 + 2 * ci
                    nc.tensor.matmul(psT[:, ci],
                                     xh[:, 64 * c0:64 * c0 + 128], id128,
                                     is_transpose=True,
                                     start=(ci == 0), stop=(ci == 1))
                # psT partition (cb, d): cb selects odd/even c
                for cb in range(2):
                    nc.vector.tensor_copy(
                        tgt[:, b4, cb],
                        psT[64 * cb:64 * cb + 64])

    def project(st):
        for nm in ("q", "k", "v"):
            for ch in range(2):
                project_unit(st, nm, ch, ptag="S")

    _qkt_cache = {}

    def _qkt_of(pair):
        qkt = _qkt_cache.get(pair)
        if qkt is None:
            qkt = ptile.tile([128, 2 * S], F16, tag="QKT", bufs=2,
                             name=f"QKT{pair}_{rep}")
            _qkt_cache[pair] = qkt
        return qkt

    def attend_load(pair):
        QKT = _qkt_of(pair)
        return QKT[:, 0:S], QKT[:, S:2 * S]

    def attend(pair, loaded, fillers=(), tail_fill=((), ()),
               vp_lazy=False):
        QT, KT = loaded
        fillers = list(fillers)
        vps = [None, None]

        def load_vp(half):
            h = 2 * pair + half
            vp = ptile.tile([128, 16, 65], F16, tag="vp", bufs=4,
                            name=f"vp{h}_{rep}")
            v_src = bass.AP(vh_d.tensor, vh_d.offset + 128 * h * E,
                            [[64, 128], [8192, 16], [1, 64]])
            with tc.high_priority():
                nc.sync.dma_start(out=vp[:, :, 0:64], in_=v_src)
            nc.vector.memset(vp[:, :, 64:65], 1.0)
            vps[half] = vp

        if not vp_lazy:
            load_vp(0)
            load_vp(1)

        LAG = 5   # defer V-matmuls 5 groups behind S^T/exp (pt bufs cover it)
        pending = []
        deferred = []   # finish_b closures, run 1-2 steps after their qc ends

        def emit_vmms(ent):
            # P^T@V with tall [128q, 65] outputs: per (j, qsub) block the PE
            # charge is 65 cols instead of 512 (M/K are free in the model).
            qc_, js_, pts_, po_ = ent
            for half in range(2):
                if vps[half] is None:
                    load_vp(half)
                pt = pts_[half]
                for jj, j in enumerate(js_):
                    for qs in range(4):
                        if 4 * qc_ + qs < j:
                            continue
                        # single psum group per (half, qc) bank: start only
                        # on the very first block, stop on the very last
                        nc.tensor.matmul(
                            po_[half][:, qs, :],
                            pt[:, 512 * jj + 128 * qs:
                               512 * jj + 128 * qs + 128],
                            vps[half][:, j, :],
                            start=(j == 0 and qs == 0),
                            stop=(j == 4 * qc_ + 3 and qs == 3))

        def finish_a(half, qc, po):
            # DVE part: rowsums -> reciprocals -> normalized [q, d] tiles.
            # Frees the po psum buffer; PE transposes come later (finish_b)
            # so the in-order PE stream never waits on this chain.
            h = 2 * pair + half
            rec = ptile.tile([128, 4, 1], F32, tag="rec", bufs=2,
                             name=f"rec{h}{qc}_{rep}")
            nc.vector.reciprocal(rec, po[:, :, 64:65])
            pn = ptile.tile([128, 4, 64], F16, tag="pn", bufs=2,
                            name=f"pn{h}{qc}_{rep}")
            # one TT with a stride-0 broadcast of rec along d
            rec_b = bass.AP(rec.tensor, rec.offset,
                            [list(d) for d in rec.ap[:2]] + [[0, 64]])
            nc.vector.tensor_tensor(pn, po[:, :, 0:64], rec_b,
                                    op=mybir.AluOpType.mult)
            return pn

        def finish_b(half, qc, pn):
            # PE transposes [128q,64d] -> [64d,128q], then strided copies
            # into the final-projection layout xT2.
            h = 2 * pair + half
            po2 = ps.tile([128, 2, 128], F16, tag="T", bufs=1,
                          name=f"po2{h}{qc}_{rep}")
            for qp in range(2):
                nc.tensor.matmul(po2[:, qp], pn[:, 2 * qp:2 * qp + 2], id128,
                                 is_transpose=True,
                                 start=(qp == 0), stop=(qp == 1))
            # po2[64qb+d, qp, ql]: query q = 128*(2qp+qb) + ql,
            # ql = 16rq + 2c2 + h2 -> xT2[64h2+d, c2, 32qc+16qp+8qb+rq]
            p_re = po2.rearrange("p qp (rq c2 h2) -> p qp h2 c2 rq",
                                 c2=8, h2=2)
            x_re = xT2[:, :, 128 * h + 32 * qc:
                       128 * h + 32 * qc + 32].rearrange(
                "p t (qp qb2 rq) -> p qp qb2 t rq", qp=2, qb2=2)
            for qb in range(2):
                for h2 in range(2):
                    nc.vector.tensor_copy(
                        x_re[64 * h2:64 * h2 + 64, :, qb],
                        p_re[64 * qb:64 * qb + 64, :, h2])

        def pop_pending():
            ent = pending.pop(0)
            emit_vmms(ent)
            qc_, js_ = ent[0], ent[1]
            if js_[-1] == 4 * qc_ + 3:   # last group of its qc
                for half in range(2):
                    pn = finish_a(half, qc_, ent[3][half])
                    deferred.append(
                        lambda half=half, qc_=qc_, pn=pn:
                        finish_b(half, qc_, pn))

        po_of = {}
        for qc in (1, 0, 2, 3):
            jmax = 4 * qc + 3
            po_of[qc] = [ps.tile([128, 4, 65], F32, tag="O", bufs=2,
                                 name=f"po{2 * pair + half}_{qc}_{rep}")
                         for half in range(2)]
            for j0 in range(0, jmax + 1, 2):
                js = [j for j in (j0, j0 + 1) if j <= jmax]
                pts = []
                for half in range(2):
                    psS = ps.tile([128, 1024], F32, tag="S", bufs=2,
                                  name=f"psS{half}_{qc}_{j0}_{rep}")
                    pt = ptile.tile([128, 1024], F16, tag="P", bufs=12,
                                    name=f"pt{half}_{qc}_{j0}_{rep}")
                    r0, r1 = 64 * half, 64 * half + 64
                    exp_runs = []    # (lo, hi) spans to exp, exact width
                    for jj, j in enumerate(js):
                        o = j - 4 * qc
                        lo = 0 if o < 0 else 128 * o
                        nc.tensor.matmul(
                            psS[:, 512 * jj + lo:512 * jj + 512],
                            KT[r0:r1, bass.ts(j, 128)],
                            QT[r0:r1, 512 * qc + lo:512 * qc + 512],
                            start=True, stop=True)
                        lo_, hi_ = 512 * jj + lo, 512 * jj + 512
                        if exp_runs and exp_runs[-1][1] == lo_:
                            exp_runs[-1] = (exp_runs[-1][0], hi_)
                        else:
                            exp_runs.append((lo_, hi_))
                    for lo_, hi_ in exp_runs:
                        nc.scalar.activation(pt[:, lo_:hi_],
                                             psS[:, lo_:hi_], EXP)
                    pts.append(pt)
                    for jj, j in enumerate(js):
                        o = j - 4 * qc
                        if o >= 0:
                            sl = pts[half][:, 512 * jj + 128 * o:
                                           512 * jj + 128 * o + 128]
                            nc.gpsimd.affine_select(
                                out=sl, in_=sl,
                                pattern=[[1, 128]],
                                compare_op=mybir.AluOpType.is_ge,
                                fill=0.0, base=0, channel_multiplier=-1)
                if fillers:
                    fillers.pop(0)()   # independent PE work while exp runs
                pending.append((qc, js, pts, po_of[qc]))
                if deferred:
                    deferred.pop(0)()
                if len(pending) > LAG:
                    pop_pending()
        # drain; run both halves' transposes before the tail final units so
        # the second half's xT2 copies overlap the first final's matmuls.
        # Leftover deferred work is handed to the caller (it becomes the
        # next pair's first fillers) so the PE stream never parks on the
        # finish chain at a pair boundary.
        while pending:
            pop_pending()
        if tail_fill != ((), ()):
            while deferred:
                deferred.pop(0)()
            for half in range(2):
                for f in tail_fill[half]:
                    f()
        for f in fillers:
            f()
        return deferred

    def final_unit(st, ch, ptag="P1"):
            # tail units borrow the psS slots (attention is over by then)
            py = ps.tile([128, 512] if ptag == "P1" else [128, 1024], F32,
                         tag=ptag, bufs=1 if ptag == "P1" else 2,
                         name=f"py{st}{ch}_{rep}")[:, 0:512]
            for t in range(8):
                nc.tensor.matmul(py,
                                 xT2[:, t, bass.ts(st, 128)],
                                 w_sb["p"][:, t, bass.ts(ch, 512)],
                                 start=(t == 0), stop=(t == 7))
            ysb = ypool.tile([128, 512], F16, tag="y",
                             name=f"ysb{st}{ch}_{rep}")
            nc.vector.tensor_tensor(ysb, py,
                                    bias_sb["p"][:, bass.ts(ch, 512)],
                                    op=mybir.AluOpType.add)
            nc.sync.dma_start(out=y[bass.ts(st, 128), bass.ts(ch, 512)],
                              in_=ysb)

    def final(st):
        for ch in range(2):
            final_unit(st, ch)

    # pipeline: proj st0/st1 dense; pair-0 attention with proj st2/st3 as
    # PE fillers; pair-1 attention with final st0/st1 as fillers; tail.
    _xh_cache.clear()
    if phases < 2:
        if load_inputs is not None:
            load_inputs("v")
        for st in range(4):
            project(st)
        return
    def _load_p():
        # demote the final-projection weight stream: it must never win the
        # DMA device or a sem slot from attention-critical transfers
        with tc.high_priority(offset=-(10 ** 6)):
            load_inputs("p")

    # q/k projections of tiles 0/1 first so pair-0 transposes start early.
    # Everything else (v, q/k st2/3) goes to fillers so the in-order PE
    # stream reaches the first scores as soon as KT(pair0) is built.
    for st, nm in ((0, "q"), (1, "q"), (0, "k"), (1, "k")):
        for ch in range(2):
            project_unit(st, nm, ch, ptag="S")
    loaded0 = attend_load(0)
    if load_inputs is not None:
        load_inputs("v")
    loaded1_box = {}
    fill0 = [
        (lambda st=st, ch=ch: project_unit(st, "v", ch))
        for st in (0, 1) for ch in range(2)
    ] + [
        (lambda st=st, nm=nm, ch=ch: project_unit(st, nm, ch))
        for nm in ("q", "k") for st in (2, 3) for ch in range(2)
    ] + [
        lambda: loaded1_box.update(v=attend_load(1)),
        (lambda: _load_p()) if load_inputs is not None else (lambda: None),
    ] + [
        (lambda st=st, ch=ch: project_unit(st, "v", ch))
        for st in (2, 3) for ch in range(2)
    ]
    left0 = attend(0, loaded0, fill0, vp_lazy=True)
    loaded1 = loaded1_box["v"]
    if phases >= 3:
        fill1 = list(left0) + [
            (lambda st=st, ch=ch: final_unit(st, ch))
            for st in (0, 1) for ch in range(2)
        ]
        tails = ([(lambda ch=ch: final_unit(2, ch, ptag="S"))
                  for ch in range(2)],
                 [(lambda ch=ch: final_unit(3, ch, ptag="S"))
                  for ch in range(2)])
    else:
        fill1, tails = [], ((), ())
    attend(1, loaded1, fill1, tail_fill=tails)


# ---------------------------------------------------------------------------
# host side
# ---------------------------------------------------------------------------

_CACHE = {}


def _prep_inputs(q, k, v, Wq, bq, Wk, bk, Wv, bv, Wp, bp):
    scale = 1.0 / np.sqrt(64.0)
    wq_T = np.ascontiguousarray(np.asarray(Wq, np.float32).T).astype(np.float16)
    wk_T = np.ascontiguousarray(np.asarray(Wk, np.float32).T).astype(np.float16)
    wv_T = np.ascontiguousarray(np.asarray(Wv, np.float32).T).astype(np.float16)
    wp_T = np.ascontiguousarray(np.asarray(Wp, np.float32).T).astype(np.float16)
    shared = {
        "wqT": wq_T, "wkT": wk_T, "wvT": wv_T, "wpT": wp_T,
        "bq": (np.asarray(bq, np.float32) * scale).astype(np.float16)[None, :],
        "bk": np.asarray(bk, np.float16)[None, :],
        "bv": np.asarray(bv, np.float16)[None, :],
        "bp": np.asarray(bp, np.float16)[None, :],
    }
    in_maps = []
    for c in range(N_CORES):
        b, g = divmod(c, 4)
        rows = slice(SB * g, SB * (g + 1))
        m = dict(shared)
        m["qT"] = np.ascontiguousarray(
            np.asarray(q[b, rows], np.float32).T * scale).astype(np.float16)
        m["kT"] = np.ascontiguousarray(
            np.asarray(k[b, rows], np.float32).T).astype(np.float16)
        m["vT"] = np.ascontiguousarray(
            np.asarray(v[b, rows], np.float32).T).astype(np.float16)
        in_maps.append(m)
    return in_maps


def kernel(q, k, v, Wq, bq, Wk, bk, Wv, bv, Wp, bp):
    if "nc" not in _CACHE:
        _CACHE["nc"] = build()
    nc = _CACHE["nc"]
    in_maps = _prep_inputs(q, k, v, Wq, bq, Wk, bk, Wv, bv, Wp, bp)
    res = run_bass_kernel_spmd(nc, in_maps, core_ids=list(range(N_CORES)))
    out = np.empty((B, S, E), np.float32)
    for c in range(N_CORES):
        b, g = divmod(c, 4)
        out[b, SB * g:SB * (g + 1), :] = res.results[c]["y"]
    return out


if __name__ == "__main__":
    rng = np.random.default_rng(0)
    s = 1.0 / np.sqrt(E)
    ins = {
        "q": rng.standard_normal((B, S, E), dtype=np.float32),
        "k": rng.standard_normal((B, S, E), dtype=np.float32),
        "v": rng.standard_normal((B, S, E), dtype=np.float32),
        "Wq": rng.standard_normal((E, E), dtype=np.float32) * s,
        "bq": rng.standard_normal(E).astype(np.float32) * s,
        "Wk": rng.standard_normal((E, E), dtype=np.float32) * s,
        "bk": rng.standard_normal(E).astype(np.float32) * s,
        "Wv": rng.standard_normal((E, E), dtype=np.float32) * s,
        "bv": rng.standard_normal(E).astype(np.float32) * s,
        "Wp": rng.standard_normal((E, E), dtype=np.float32) * s,
        "bp": rng.standard_normal(E).astype(np.float32) * s,
    }
    out = kernel(**ins)
    print("kernel ran, out shape", out.shape, "mean", float(np.abs(out).mean()))



# revision 78
# speedup vs baseline: 1.2009x; 1.0078x over previous
"""Trainium2 Bass kernel for nn_MultiHeadAttention_66202625900642.

Reference semantics (B=2, S=2048, E=1024, H=16 heads, D=64):
    qh = q @ Wq.T + bq   (same k, v)
    head split is a PLAIN RESHAPE (B, S, E) -> (B, H, S, D):
      head h of batch b = rows [128h, 128h+128) of qh[b] reinterpreted
      row-major as a (2048, 64) matrix (scrambled seq index s' = 16r + c).
    causal softmax over s', out @ Wp.T + bp.

Because the head split partitions the *sequence* rows, sharding each batch
into 4 row-blocks of 512 (= 4 heads) is fully local: 8 cores = 2 batches x 4
quarters, zero collectives. Weights are replicated (fp16).

Per-core pipeline (all matmuls fp16, fp32 PSUM accumulation; tuned against
the TimelineSim cost model: matmul cost = output columns only, engine-op
cost = free-dim size with free strides, serial DMA device, 2KB psum zero
regions, PE p-state ramp):
  1. projections -> qh/kh/vh fp16; inputs/weights stream in fine-grained
     t-interleaved chunks on the SP hwdge queue (ACT's sequencer stays free
     for exp dispatch); bias adds ride the psum->sbuf evacuation on DVE.
  2. Q^T/K^T built in SBUF by 128x128 PE transposes (identity matmuls) +
     strided DVE copies - no DRAM round-trip or DMA-transposes. vh goes
     through DRAM and is re-read as [128, 16, 65] V' tiles (ones column ->
     rowsums ride the P^T @ V matmul).
  3. attention per head pair with a rolling software pipeline: per 2-key-
     block step, S^T matmuls (both heads on disjoint d-row groups), exact-
     span exps on ACT, causal triangles via gpsimd affine_select; P^T @ V
     runs LAG=5 steps behind with tall [128q, 4qs, 65] psum accumulation
     (single psum group per bank); projection/final-projection units are
     interleaved as PE fillers.
  4. per-chunk finish, split so the PE stream never parks: DVE reciprocal
     + one broadcast TT normalizes [q, d] tiles; deferred PE transposes +
     strided copies scatter into the final-projection layout (leftover
     deferred work becomes the next pair's first fillers).
  5. final projection -> y fp16 (host upcasts to f32).
"""

import numpy as np

import concourse.bass as bass
import concourse.mybir as mybir
import concourse.tile as tile
from concourse import bacc
from concourse.bass_utils import run_bass_kernel_spmd

F16 = mybir.dt.float16
F32 = mybir.dt.float32
F32R = mybir.dt.float32r
EXP = mybir.ActivationFunctionType.Exp

B, S, E = 2, 2048, 1024
SB = 512                # seq rows per core (= 4 heads)
N_CORES = 8


def build(reps: int = 1, phases: int = 3):
    nc = bacc.Bacc(None, target_bir_lowering=False)

    qT = nc.dram_tensor("qT", [E, SB], F16, kind="ExternalInput")
    kT = nc.dram_tensor("kT", [E, SB], F16, kind="ExternalInput")
    vT = nc.dram_tensor("vT", [E, SB], F16, kind="ExternalInput")
    wqT = nc.dram_tensor("wqT", [E, E], F16, kind="ExternalInput")
    wkT = nc.dram_tensor("wkT", [E, E], F16, kind="ExternalInput")
    wvT = nc.dram_tensor("wvT", [E, E], F16, kind="ExternalInput")
    wpT = nc.dram_tensor("wpT", [E, E], F16, kind="ExternalInput")
    bq = nc.dram_tensor("bq", [1, E], F16, kind="ExternalInput")
    bk = nc.dram_tensor("bk", [1, E], F16, kind="ExternalInput")
    bv = nc.dram_tensor("bv", [1, E], F16, kind="ExternalInput")
    bp = nc.dram_tensor("bp", [1, E], F16, kind="ExternalInput")
    y = nc.dram_tensor("y", [SB, E], F16, kind="ExternalOutput")

    with tile.TileContext(nc) as tc:
        with (
            tc.tile_pool(name="consts", bufs=1) as consts,
            tc.tile_pool(name="wpool", bufs=1) as wpool,
            tc.tile_pool(name="proj", bufs=2) as proj,
            tc.tile_pool(name="attn", bufs=1) as attn,
            tc.tile_pool(name="ptile", bufs=3) as ptile,
            tc.tile_pool(name="ypool", bufs=2) as ypool,
            tc.tile_pool(name="ps", bufs=3, space="PSUM") as ps,
            tc.tile_pool(name="dram", bufs=1, space="DRAM") as dram,
        ):
            # ---- constants -------------------------------------------------
            ones128 = consts.tile([1, 128], F16)
            nc.vector.memset(ones128, 1.0)
            # identity for PE tile transposes
            id128 = consts.tile([128, 128], F16)
            nc.vector.memset(id128, 1.0)
            nc.gpsimd.affine_select(
                out=id128, in_=id128, pattern=[[1, 128]],
                compare_op=mybir.AluOpType.is_equal,
                fill=0.0, base=0, channel_multiplier=-1)
            bias_sb = {}
            for nm, t in (("q", bq), ("k", bk), ("v", bv), ("p", bp)):
                b_t = consts.tile([1, E], F16, name=f"bias_{nm}")
                nc.sync.dma_start(out=b_t, in_=t[:, :])
                # bias adds ride the psum->sbuf evacuation on DVE instead of
                # burning PE columns: broadcast each to all 128 partitions.
                b_bc = consts.tile([128, E], F16, name=f"biasbc_{nm}")
                nc.gpsimd.partition_broadcast(b_bc, b_t)
                bias_sb[nm] = b_bc

            # ---- weight/activation tiles; q/k loaded now, v/p deferred -----
            w_sb, x_sb, dram_in = {}, {}, {}
            for nm, wt, xt in (("q", wqT, qT), ("k", wkT, kT), ("v", wvT, vT)):
                w_t = wpool.tile([128, 8, E], F16, name=f"w_{nm}")
                x_t = wpool.tile([128, 8, SB], F16, name=f"x_{nm}")
                dram_in[nm] = (wt, xt)
                w_sb[nm], x_sb[nm] = w_t, x_t
            w_p = wpool.tile([128, 8, E], F16, name="w_p")
            w_sb["p"] = w_p
            dram_in["p"] = (wpT, None)

            def load_inputs(nm, eng=None, part="all"):
                # fine-grained, t-interleaved chunks so the first projection
                # matmul starts after ~1MB instead of ~3MB (DMA device is
                # serial in the cost model). SP queue: ACT's sequencer must
                # stay free for exp dispatch (no exec queue on ACT).
                # part="lo" ships only the first seq half of x (enough for
                # st0/st1, i.e. the pair-0 critical path); "hi" ships the
                # rest for the st2/st3 filler units.
                eng = eng or nc.sync
                wt, xt = dram_in[nm]
                wre = wt.ap().rearrange("(t p) f -> p t f", p=128)
                if xt is not None:
                    xre = xt.ap().rearrange("(t p) s -> p t s", p=128)
                    for t4 in range(2):
                        if part == "hi":
                            eng.dma_start(
                                out=x_sb[nm][:, 4 * t4:4 * t4 + 4, 256:512],
                                in_=xre[:, 4 * t4:4 * t4 + 4, 256:512])
                            continue
                        sl = slice(0, 256) if part == "lo" else slice(0, 512)
                        eng.dma_start(out=x_sb[nm][:, 4 * t4:4 * t4 + 4, sl],
                                      in_=xre[:, 4 * t4:4 * t4 + 4, sl])
                        for t in range(4 * t4, 4 * t4 + 4):
                            eng.dma_start(out=w_sb[nm][:, t:t + 1],
                                          in_=wre[:, t:t + 1])
                else:
                    for t2 in range(4):
                        eng.dma_start(out=w_sb[nm][:, 2 * t2:2 * t2 + 2],
                                      in_=wre[:, 2 * t2:2 * t2 + 2])

            load_inputs("q", part="lo")
            load_inputs("k", part="lo")
            load_inputs("q", part="hi")
            load_inputs("k", part="hi")

            # ---- DRAM scratch ---------------------------------------------
            vh_d = dram.tile([SB, E], F16)

            for rep in range(reps):
                _body(nc, tc, ps, proj, attn, ptile, ypool,
                      ones128, id128, bias_sb, w_sb, x_sb, vh_d, y,
                      rep, phases, load_inputs if rep == 0 else None)
    nc.finalize()
    return nc


def _body(nc, tc, ps, proj, attn, ptile, ypool, ones128, id128,
          bias_sb, w_sb, x_sb, vh_d, y, rep, phases=3,
          load_inputs=None):
    xT2 = attn.tile([128, 8, SB], F16, tag="xT2", name=f"xT2_{rep}")
    if phases < 2:
        nc.vector.memset(xT2[:, 0, 0:1], 0.0)
    _xh_cache = {}

    def project_unit(st, nm, ch, ptag="P1"):
        # one psum-group of the projection for (seq-tile st, proj nm, chunk ch)
        xh = _xh_cache.get((st, nm))
        if xh is None:
            xh = proj.tile([128, E], F16, tag="xh", name=f"xh_{nm}{st}_{rep}")
            _xh_cache[(st, nm)] = xh
        # initial-phase units borrow the (then idle) psS slots so the filler
        # tag P1 can stay at one buffer (PSUM is fully subscribed)
        pp = ps.tile([128, 512] if ptag == "P1" else [128, 1024], F32,
                     tag=ptag, bufs=1 if ptag == "P1" else 2,
                     name=f"pp{rep}")[:, 0:512]
        for t in range(8):
            nc.tensor.matmul(
                pp,
                x_sb[nm][:, t, bass.ts(st, 128)],
                w_sb[nm][:, t, bass.ts(ch, 512)],
                start=(t == 0), stop=(t == 7))
        nc.vector.tensor_tensor(xh[:, bass.ts(ch, 512)], pp,
                                bias_sb[nm][:, bass.ts(ch, 512)],
                                op=mybir.AluOpType.add)
        if nm == "v":
            if ch == 1:
                nc.sync.dma_start(out=vh_d[bass.ts(st, 128), :], in_=xh)
        else:
            # Q^T/K^T built in SBUF via PE transposes + strided DVE copies:
            # no DRAM round-trip, no DMA-transpose, no cross-queue semaphore
            # coupling. xh cols (c,d) -> QKT[64h2+d, 16p+c]. Emitted per ch
            # (each ch covers half the c's) to shorten the critical chain.
            qkt = _qkt_of(st // 2)
            half = st % 2
            off = 0 if nm == "q" else S
            tgt = qkt[64 * half:64 * half + 64, off:off + S].rearrange(
                "p (q b c2 cb) -> p b cb c2 q", b=4, c2=2, cb=2)
            for b4 in (0, 1) if ch == 0 else (2, 3):
                # 128x128 transposes: two c-columns per pass (PE charge
                # is output cols only), alternating psum tags for depth
                ptag2, pbufs = ("T", 1) if b4 % 2 == 0 else ("O", 2)
                psT = ps.tile([128, 2, 128], F16, tag=ptag2, bufs=pbufs,
                              name=f"psT{nm}{st}{b4}_{rep}")
                for ci in range(2):
                    c0 = 4 * b4 + 2 * ci
                    nc.tensor.matmul(psT[:, ci],
                                     xh[:, 64 * c0:64 * c0 + 128], id128,
                                     is_transpose=True,
                                     start=(ci == 0), stop=(ci == 1))
                # psT partition (cb, d): cb selects odd/even c
                for cb in range(2):
                    nc.vector.tensor_copy(
                        tgt[:, b4, cb],
                        psT[64 * cb:64 * cb + 64])

    def project(st):
        for nm in ("q", "k", "v"):
            for ch in range(2):
                project_unit(st, nm, ch, ptag="S")

    _qkt_cache = {}

    def _qkt_of(pair):
        qkt = _qkt_cache.get(pair)
        if qkt is None:
            qkt = ptile.tile([128, 2 * S], F16, tag="QKT", bufs=2,
                             name=f"QKT{pair}_{rep}")
            _qkt_cache[pair] = qkt
        return qkt

    def attend_load(pair):
        QKT = _qkt_of(pair)
        return QKT[:, 0:S], QKT[:, S:2 * S]

    def attend(pair, loaded, fillers=(), tail_fill=((), ()),
               vp_lazy=False):
        QT, KT = loaded
        fillers = list(fillers)
        vps = [None, None]

        def load_vp(half):
            h = 2 * pair + half
            vp = ptile.tile([128, 16, 65], F16, tag="vp", bufs=4,
                            name=f"vp{h}_{rep}")
            v_src = bass.AP(vh_d.tensor, vh_d.offset + 128 * h * E,
                            [[64, 128], [8192, 16], [1, 64]])
            with tc.high_priority():
                nc.sync.dma_start(out=vp[:, :, 0:64], in_=v_src)
            nc.vector.memset(vp[:, :, 64:65], 1.0)
            vps[half] = vp

        if not vp_lazy:
            load_vp(0)
            load_vp(1)

        LAG = 5   # defer V-matmuls 5 groups behind S^T/exp (pt bufs cover it)
        pending = []
        deferred = []   # finish_b closures, run 1-2 steps after their qc ends

        def emit_vmms(ent):
            # P^T@V with tall [128q, 65] outputs: per (j, qsub) block the PE
            # charge is 65 cols instead of 512 (M/K are free in the model).
            qc_, js_, pts_, po_ = ent
            for half, pt in pts_.items():
                if vps[half] is None:
                    load_vp(half)
                for jj, j in enumerate(js_):
                    for qs in range(4):
                        if 4 * qc_ + qs < j:
                            continue
                        # single psum group per (half, qc) bank: start only
                        # on the very first block, stop on the very last
                        nc.tensor.matmul(
                            po_[half][:, qs, :],
                            pt[:, 512 * jj + 128 * qs:
                               512 * jj + 128 * qs + 128],
                            vps[half][:, j, :],
                            start=(j == 0 and qs == 0),
                            stop=(j == 4 * qc_ + 3 and qs == 3))

        def finish_a(half, qc, po):
            # DVE part: rowsums -> reciprocals -> normalized [q, d] tiles.
            # Frees the po psum buffer; PE transposes come later (finish_b)
            # so the in-order PE stream never waits on this chain.
            h = 2 * pair + half
            rec = ptile.tile([128, 4, 1], F32, tag="rec", bufs=2,
                             name=f"rec{h}{qc}_{rep}")
            nc.vector.reciprocal(rec, po[:, :, 64:65])
            pn = ptile.tile([128, 4, 64], F16, tag="pn", bufs=2,
                            name=f"pn{h}{qc}_{rep}")
            # one TT with a stride-0 broadcast of rec along d
            rec_b = bass.AP(rec.tensor, rec.offset,
                            [list(d) for d in rec.ap[:2]] + [[0, 64]])
            nc.vector.tensor_tensor(pn, po[:, :, 0:64], rec_b,
                                    op=mybir.AluOpType.mult)
            return pn

        def finish_b(half, qc, pn):
            # PE transposes [128q,64d] -> [64d,128q], then strided copies
            # into the final-projection layout xT2.
            h = 2 * pair + half
            po2 = ps.tile([128, 2, 128], F16, tag="T", bufs=1,
                          name=f"po2{h}{qc}_{rep}")
            for qp in range(2):
                nc.tensor.matmul(po2[:, qp], pn[:, 2 * qp:2 * qp + 2], id128,
                                 is_transpose=True,
                                 start=(qp == 0), stop=(qp == 1))
            # po2[64qb+d, qp, ql]: query q = 128*(2qp+qb) + ql,
            # ql = 16rq + 2c2 + h2 -> xT2[64h2+d, c2, 32qc+16qp+8qb+rq]
            p_re = po2.rearrange("p qp (rq c2 h2) -> p qp h2 c2 rq",
                                 c2=8, h2=2)
            x_re = xT2[:, :, 128 * h + 32 * qc:
                       128 * h + 32 * qc + 32].rearrange(
                "p t (qp qb2 rq) -> p qp qb2 t rq", qp=2, qb2=2)
            for qb in range(2):
                for h2 in range(2):
                    nc.vector.tensor_copy(
                        x_re[64 * h2:64 * h2 + 64, :, qb],
                        p_re[64 * qb:64 * qb + 64, :, h2])
            if qc == 3:
                for f in tail_fill[half]:
                    f()

        def pop_pending():
            ent = pending.pop(0)
            emit_vmms(ent)
            qc_, js_ = ent[0], ent[1]
            if js_[-1] == 4 * qc_ + 3:   # last group of its qc
                for half in ent[2]:
                    pn = finish_a(half, qc_, ent[3][half])
                    deferred.append(
                        lambda half=half, qc_=qc_, pn=pn:
                        finish_b(half, qc_, pn))

        def emit_half_step(qc, js, half):
            psS = ps.tile([128, 1024], F32, tag="S", bufs=2,
                          name=f"psS{half}_{qc}_{js[0]}_{rep}")
            pt = ptile.tile([128, 1024], F16, tag="P", bufs=12,
                            name=f"pt{half}_{qc}_{js[0]}_{rep}")
            r0, r1 = 64 * half, 64 * half + 64
            exp_runs = []    # (lo, hi) spans to exp, exact width
            for jj, j in enumerate(js):
                o = j - 4 * qc
                lo = 0 if o < 0 else 128 * o
                nc.tensor.matmul(
                    psS[:, 512 * jj + lo:512 * jj + 512],
                    KT[r0:r1, bass.ts(j, 128)],
                    QT[r0:r1, 512 * qc + lo:512 * qc + 512],
                    start=True, stop=True)
                lo_, hi_ = 512 * jj + lo, 512 * jj + 512
                if exp_runs and exp_runs[-1][1] == lo_:
                    exp_runs[-1] = (exp_runs[-1][0], hi_)
                else:
                    exp_runs.append((lo_, hi_))
            for lo_, hi_ in exp_runs:
                nc.scalar.activation(pt[:, lo_:hi_], psS[:, lo_:hi_], EXP)
            for jj, j in enumerate(js):
                o = j - 4 * qc
                if o >= 0:
                    sl = pt[:, 512 * jj + 128 * o:512 * jj + 128 * o + 128]
                    nc.gpsimd.affine_select(
                        out=sl, in_=sl,
                        pattern=[[1, 128]],
                        compare_op=mybir.AluOpType.is_ge,
                        fill=0.0, base=0, channel_multiplier=-1)
            return pt

        def step_tail():
            if fillers:
                fillers.pop(0)()   # independent PE work while exp runs
            if deferred:
                deferred.pop(0)()
            if len(pending) > LAG:
                pop_pending()

        po_of = {}
        for qc in (1, 0, 2, 3):
            jmax = 4 * qc + 3
            po_of[qc] = [ps.tile([128, 4, 65], F32, tag="O", bufs=2,
                                 name=f"po{2 * pair + half}_{qc}_{rep}")
                         for half in range(2)]
            if qc != 3 or pair == 0:
                for j0 in range(0, jmax + 1, 2):
                    js = [j0, j0 + 1]
                    pts = {half: emit_half_step(qc, js, half)
                           for half in range(2)}
                    pending.append((qc, js, pts, po_of[qc]))
                    step_tail()
            else:
                # last chunk: halves staggered so the first head's finish +
                # final projection overlap the second head's exps
                for half in range(2):
                    for j0 in range(0, jmax + 1, 2):
                        js = [j0, j0 + 1]
                        pts = {half: emit_half_step(qc, js, half)}
                        pending.append((qc, js, pts, po_of[qc]))
                        step_tail()
        # drain. Leftover deferred work is handed to the caller (it becomes
        # the next pair's first fillers) so the PE stream never parks on the
        # finish chain at a pair boundary.
        while pending:
            pop_pending()
        if tail_fill != ((), ()):
            while deferred:
                deferred.pop(0)()
        for f in fillers:
            f()
        return deferred

    def final_unit(st, ch, ptag="P1"):
            # tail units borrow the psS slots (attention is over by then)
            py = ps.tile([128, 512] if ptag == "P1" else [128, 1024], F32,
                         tag=ptag, bufs=1 if ptag == "P1" else 2,
                         name=f"py{st}{ch}_{rep}")[:, 0:512]
            for t in range(8):
                nc.tensor.matmul(py,
                                 xT2[:, t, bass.ts(st, 128)],
                                 w_sb["p"][:, t, bass.ts(ch, 512)],
                                 start=(t == 0), stop=(t == 7))
            ysb = ypool.tile([128, 512], F16, tag="y",
                             name=f"ysb{st}{ch}_{rep}")
            nc.vector.tensor_tensor(ysb, py,
                                    bias_sb["p"][:, bass.ts(ch, 512)],
                                    op=mybir.AluOpType.add)
            nc.sync.dma_start(out=y[bass.ts(st, 128), bass.ts(ch, 512)],
                              in_=ysb)

    def final(st):
        for ch in range(2):
            final_unit(st, ch)

    # pipeline: proj st0/st1 dense; pair-0 attention with proj st2/st3 as
    # PE fillers; pair-1 attention with final st0/st1 as fillers; tail.
    _xh_cache.clear()
    if phases < 2:
        if load_inputs is not None:
            load_inputs("v")
        for st in range(4):
            project(st)
        return
    def _load_p():
        # demote the final-projection weight stream: it must never win the
        # DMA device or a sem slot from attention-critical transfers
        with tc.high_priority(offset=-(10 ** 6)):
            load_inputs("p")

    # q/k projections of tiles 0/1 first so pair-0 transposes start early.
    # Everything else (v, q/k st2/3) goes to fillers so the in-order PE
    # stream reaches the first scores as soon as KT(pair0) is built.
    for st, nm in ((0, "q"), (1, "q"), (0, "k"), (1, "k")):
        for ch in range(2):
            project_unit(st, nm, ch, ptag="S")
    loaded0 = attend_load(0)
    if load_inputs is not None:
        load_inputs("v")
    loaded1_box = {}
    fill0 = [
        (lambda st=st, ch=ch: project_unit(st, "v", ch))
        for st in (0, 1) for ch in range(2)
    ] + [
        (lambda st=st, nm=nm, ch=ch: project_unit(st, nm, ch))
        for nm in ("q", "k") for st in (2, 3) for ch in range(2)
    ] + [
        lambda: loaded1_box.update(v=attend_load(1)),
        (lambda: _load_p()) if load_inputs is not None else (lambda: None),
    ] + [
        (lambda st=st, ch=ch: project_unit(st, "v", ch))
        for st in (2, 3) for ch in range(2)
    ]
    left0 = attend(0, loaded0, fill0, vp_lazy=True)
    loaded1 = loaded1_box["v"]
    if phases >= 3:
        fill1 = list(left0) + [
            (lambda st=st, ch=ch: final_unit(st, ch))
            for st in (0, 1) for ch in range(2)
        ]
        tails = ([(lambda ch=ch: final_unit(2, ch, ptag="S"))
                  for ch in range(2)],
                 [(lambda ch=ch: final_unit(3, ch, ptag="S"))
                  for ch in range(2)])
    else:
        fill1, tails = [], ((), ())
    attend(1, loaded1, fill1, tail_fill=tails)


# ---------------------------------------------------------------------------
# host side
# ---------------------------------------------------------------------------

_CACHE = {}


def _prep_inputs(q, k, v, Wq, bq, Wk, bk, Wv, bv, Wp, bp):
    scale = 1.0 / np.sqrt(64.0)
    wq_T = np.ascontiguousarray(np.asarray(Wq, np.float32).T).astype(np.float16)
    wk_T = np.ascontiguousarray(np.asarray(Wk, np.float32).T).astype(np.float16)
    wv_T = np.ascontiguousarray(np.asarray(Wv, np.float32).T).astype(np.float16)
    wp_T = np.ascontiguousarray(np.asarray(Wp, np.float32).T).astype(np.float16)
    shared = {
        "wqT": wq_T, "wkT": wk_T, "wvT": wv_T, "wpT": wp_T,
        "bq": (np.asarray(bq, np.float32) * scale).astype(np.float16)[None, :],
        "bk": np.asarray(bk, np.float16)[None, :],
        "bv": np.asarray(bv, np.float16)[None, :],
        "bp": np.asarray(bp, np.float16)[None, :],
    }
    in_maps = []
    for c in range(N_CORES):
        b, g = divmod(c, 4)
        rows = slice(SB * g, SB * (g + 1))
        m = dict(shared)
        m["qT"] = np.ascontiguousarray(
            np.asarray(q[b, rows], np.float32).T * scale).astype(np.float16)
        m["kT"] = np.ascontiguousarray(
            np.asarray(k[b, rows], np.float32).T).astype(np.float16)
        m["vT"] = np.ascontiguousarray(
            np.asarray(v[b, rows], np.float32).T).astype(np.float16)
        in_maps.append(m)
    return in_maps


def kernel(q, k, v, Wq, bq, Wk, bk, Wv, bv, Wp, bp):
    if "nc" not in _CACHE:
        _CACHE["nc"] = build()
    nc = _CACHE["nc"]
    in_maps = _prep_inputs(q, k, v, Wq, bq, Wk, bk, Wv, bv, Wp, bp)
    res = run_bass_kernel_spmd(nc, in_maps, core_ids=list(range(N_CORES)))
    out = np.empty((B, S, E), np.float32)
    for c in range(N_CORES):
        b, g = divmod(c, 4)
        out[b, SB * g:SB * (g + 1), :] = res.results[c]["y"]
    return out


if __name__ == "__main__":
    rng = np.random.default_rng(0)
    s = 1.0 / np.sqrt(E)
    ins = {
        "q": rng.standard_normal((B, S, E), dtype=np.float32),
        "k": rng.standard_normal((B, S, E), dtype=np.float32),
        "v": rng.standard_normal((B, S, E), dtype=np.float32),
        "Wq": rng.standard_normal((E, E), dtype=np.float32) * s,
        "bq": rng.standard_normal(E).astype(np.float32) * s,
        "Wk": rng.standard_normal((E, E), dtype=np.float32) * s,
        "bk": rng.standard_normal(E).astype(np.float32) * s,
        "Wv": rng.standard_normal((E, E), dtype=np.float32) * s,
        "bv": rng.standard_normal(E).astype(np.float32) * s,
        "Wp": rng.standard_normal((E, E), dtype=np.float32) * s,
        "bp": rng.standard_normal(E).astype(np.float32) * s,
    }
    out = kernel(**ins)
    print("kernel ran, out shape", out.shape, "mean", float(np.abs(out).mean()))



# revision 82
# speedup vs baseline: 1.2288x; 1.0232x over previous
"""Trainium2 Bass kernel for nn_MultiHeadAttention_66202625900642.

Reference semantics (B=2, S=2048, E=1024, H=16 heads, D=64):
    qh = q @ Wq.T + bq   (same k, v)
    head split is a PLAIN RESHAPE (B, S, E) -> (B, H, S, D):
      head h of batch b = rows [128h, 128h+128) of qh[b] reinterpreted
      row-major as a (2048, 64) matrix (scrambled seq index s' = 16r + c).
    causal softmax over s', out @ Wp.T + bp.

Because the head split partitions the *sequence* rows, sharding each batch
into 4 row-blocks of 512 (= 4 heads) is fully local: 8 cores = 2 batches x 4
quarters, zero collectives. Weights are replicated (fp16).

Per-core pipeline (all matmuls fp16, fp32 PSUM accumulation; tuned against
the TimelineSim cost model: matmul cost = output columns only, engine-op
cost = free-dim size with free strides, serial DMA device, 2KB psum zero
regions, PE p-state ramp):
  1. projections -> qh/kh/vh fp16; inputs/weights stream in fine-grained
     t-interleaved chunks on the SP hwdge queue (ACT's sequencer stays free
     for exp dispatch); bias adds ride the psum->sbuf evacuation on DVE.
  2. Q^T/K^T built in SBUF by 128x128 PE transposes (identity matmuls) +
     strided DVE copies - no DRAM round-trip or DMA-transposes. vh goes
     through DRAM and is re-read as [128, 16, 65] V' tiles (ones column ->
     rowsums ride the P^T @ V matmul).
  3. attention per head pair with a rolling software pipeline: per 2-key-
     block step, S^T matmuls (both heads on disjoint d-row groups), exact-
     span exps on ACT, causal triangles via gpsimd affine_select; P^T @ V
     runs LAG=5 steps behind with tall [128q, 4qs, 65] psum accumulation
     (single psum group per bank); projection/final-projection units are
     interleaved as PE fillers.
  4. per-chunk finish, split so the PE stream never parks: DVE reciprocal
     + one broadcast TT normalizes [q, d] tiles; deferred PE transposes +
     strided copies scatter into the final-projection layout (leftover
     deferred work becomes the next pair's first fillers).
  5. final projection -> y fp16 (host upcasts to f32).
"""

import numpy as np

import concourse.bass as bass
import concourse.mybir as mybir
import concourse.tile as tile
from concourse import bacc
from concourse.bass_utils import run_bass_kernel_spmd

F16 = mybir.dt.float16
F32 = mybir.dt.float32
F32R = mybir.dt.float32r
EXP = mybir.ActivationFunctionType.Exp

B, S, E = 2, 2048, 1024
SB = 512                # seq rows per core (= 4 heads)
N_CORES = 8


def build(reps: int = 1, phases: int = 3):
    nc = bacc.Bacc(None, target_bir_lowering=False)

    qT = nc.dram_tensor("qT", [E, SB], F16, kind="ExternalInput")
    kT = nc.dram_tensor("kT", [E, SB], F16, kind="ExternalInput")
    vT = nc.dram_tensor("vT", [E, SB], F16, kind="ExternalInput")
    wqT = nc.dram_tensor("wqT", [E, E], F16, kind="ExternalInput")
    wkT = nc.dram_tensor("wkT", [E, E], F16, kind="ExternalInput")
    wvT = nc.dram_tensor("wvT", [E, E], F16, kind="ExternalInput")
    wpT = nc.dram_tensor("wpT", [E, E], F16, kind="ExternalInput")
    bq = nc.dram_tensor("bq", [1, E], F16, kind="ExternalInput")
    bk = nc.dram_tensor("bk", [1, E], F16, kind="ExternalInput")
    bv = nc.dram_tensor("bv", [1, E], F16, kind="ExternalInput")
    bp = nc.dram_tensor("bp", [1, E], F16, kind="ExternalInput")
    y = nc.dram_tensor("y", [SB, E], F16, kind="ExternalOutput")

    with tile.TileContext(nc) as tc:
        with (
            tc.tile_pool(name="consts", bufs=1) as consts,
            tc.tile_pool(name="wpool", bufs=1) as wpool,
            tc.tile_pool(name="proj", bufs=2) as proj,
            tc.tile_pool(name="attn", bufs=1) as attn,
            tc.tile_pool(name="ptile", bufs=3) as ptile,
            tc.tile_pool(name="ypool", bufs=2) as ypool,
            tc.tile_pool(name="ps", bufs=3, space="PSUM") as ps,
            tc.tile_pool(name="dram", bufs=1, space="DRAM") as dram,
        ):
            # ---- constants -------------------------------------------------
            ones128 = consts.tile([1, 128], F16)
            nc.vector.memset(ones128, 1.0)
            # identity for PE tile transposes
            id128 = consts.tile([128, 128], F16)
            nc.vector.memset(id128, 1.0)
            nc.gpsimd.affine_select(
                out=id128, in_=id128, pattern=[[1, 128]],
                compare_op=mybir.AluOpType.is_equal,
                fill=0.0, base=0, channel_multiplier=-1)
            bias_sb = {}
            for nm, t in (("q", bq), ("k", bk), ("v", bv), ("p", bp)):
                b_t = consts.tile([1, E], F16, name=f"bias_{nm}")
                nc.sync.dma_start(out=b_t, in_=t[:, :])
                # bias adds ride the psum->sbuf evacuation on DVE instead of
                # burning PE columns: broadcast each to all 128 partitions.
                b_bc = consts.tile([128, E], F16, name=f"biasbc_{nm}")
                nc.gpsimd.partition_broadcast(b_bc, b_t)
                bias_sb[nm] = b_bc

            # ---- weight/activation tiles; q/k loaded now, v/p deferred -----
            w_sb, x_sb, dram_in = {}, {}, {}
            for nm, wt, xt in (("q", wqT, qT), ("k", wkT, kT), ("v", wvT, vT)):
                w_t = wpool.tile([128, 8, E], F16, name=f"w_{nm}")
                x_t = wpool.tile([128, 8, SB], F16, name=f"x_{nm}")
                dram_in[nm] = (wt, xt)
                w_sb[nm], x_sb[nm] = w_t, x_t
            w_p = wpool.tile([128, 8, E], F16, name="w_p")
            w_sb["p"] = w_p
            dram_in["p"] = (wpT, None)

            def load_inputs(nm, eng=None, part="all"):
                # fine-grained, t-interleaved chunks so the first projection
                # matmul starts after ~1MB instead of ~3MB (DMA device is
                # serial in the cost model). SP queue: ACT's sequencer must
                # stay free for exp dispatch (no exec queue on ACT).
                # part="lo" ships only the first seq half of x (enough for
                # st0/st1, i.e. the pair-0 critical path); "hi" ships the
                # rest for the st2/st3 filler units.
                eng = eng or nc.sync
                wt, xt = dram_in[nm]
                wre = wt.ap().rearrange("(t p) f -> p t f", p=128)
                if xt is not None:
                    xre = xt.ap().rearrange("(t p) s -> p t s", p=128)
                    for t4 in range(2):
                        if part == "hi":
                            eng.dma_start(
                                out=x_sb[nm][:, 4 * t4:4 * t4 + 4, 256:512],
                                in_=xre[:, 4 * t4:4 * t4 + 4, 256:512])
                            continue
                        sl = slice(0, 256) if part == "lo" else slice(0, 512)
                        eng.dma_start(out=x_sb[nm][:, 4 * t4:4 * t4 + 4, sl],
                                      in_=xre[:, 4 * t4:4 * t4 + 4, sl])
                        for t in range(4 * t4, 4 * t4 + 4):
                            eng.dma_start(out=w_sb[nm][:, t:t + 1],
                                          in_=wre[:, t:t + 1])
                else:
                    for t2 in range(4):
                        eng.dma_start(out=w_sb[nm][:, 2 * t2:2 * t2 + 2],
                                      in_=wre[:, 2 * t2:2 * t2 + 2])

            load_inputs("q", part="lo")
            load_inputs("k", part="lo")
            load_inputs("q", part="hi")
            load_inputs("k", part="hi")

            # ---- DRAM scratch ---------------------------------------------
            vh_d = dram.tile([SB, E], F16)

            for rep in range(reps):
                _body(nc, tc, ps, proj, attn, ptile, ypool,
                      ones128, id128, bias_sb, w_sb, x_sb, vh_d, y,
                      rep, phases, load_inputs if rep == 0 else None)
    nc.finalize()
    return nc


def _body(nc, tc, ps, proj, attn, ptile, ypool, ones128, id128,
          bias_sb, w_sb, x_sb, vh_d, y, rep, phases=3,
          load_inputs=None):
    xT2 = attn.tile([128, 8, SB], F16, tag="xT2", name=f"xT2_{rep}")
    if phases < 2:
        nc.vector.memset(xT2[:, 0, 0:1], 0.0)
    _xh_cache = {}

    def project_unit(st, nm, ch, ptag="P1"):
        # one psum-group of the projection for (seq-tile st, proj nm, chunk ch)
        xh = _xh_cache.get((st, nm))
        if xh is None:
            xh = proj.tile([128, E], F16, tag="xh", name=f"xh_{nm}{st}_{rep}")
            _xh_cache[(st, nm)] = xh
        # initial-phase units borrow the (then idle) psS slots so the filler
        # tag P1 can stay at one buffer (PSUM is fully subscribed)
        pp = ps.tile([128, 512] if ptag == "P1" else [128, 1024], F32,
                     tag=ptag, bufs=1 if ptag == "P1" else 2,
                     name=f"pp{rep}")[:, 0:512]
        for t in range(8):
            nc.tensor.matmul(
                pp,
                x_sb[nm][:, t, bass.ts(st, 128)],
                w_sb[nm][:, t, bass.ts(ch, 512)],
                start=(t == 0), stop=(t == 7))
        nc.vector.tensor_tensor(xh[:, bass.ts(ch, 512)], pp,
                                bias_sb[nm][:, bass.ts(ch, 512)],
                                op=mybir.AluOpType.add)
        if nm == "v":
            if ch == 1:
                nc.sync.dma_start(out=vh_d[bass.ts(st, 128), :], in_=xh)
        else:
            # Q^T/K^T built in SBUF via PE transposes + strided DVE copies:
            # no DRAM round-trip, no DMA-transpose, no cross-queue semaphore
            # coupling. xh cols (c,d) -> QKT[64h2+d, 16p+c]. Emitted per ch
            # (each ch covers half the c's) to shorten the critical chain.
            qkt = _qkt_of(st // 2)
            half = st % 2
            off = 0 if nm == "q" else S
            tgt = qkt[64 * half:64 * half + 64, off:off + S].rearrange(
                "p (q b c2 cb) -> p b cb c2 q", b=4, c2=2, cb=2)
            for b4 in (0, 1) if ch == 0 else (2, 3):
                # 128x128 transposes: two c-columns per pass (PE charge
                # is output cols only), alternating psum tags for depth
                ptag2, pbufs = ("T", 1) if b4 % 2 == 0 else ("O", 2)
                psT = ps.tile([128, 2, 128], F16, tag=ptag2, bufs=pbufs,
                              name=f"psT{nm}{st}{b4}_{rep}")
                for ci in range(2):
                    c0 = 4 * b4 + 2 * ci
                    nc.tensor.matmul(psT[:, ci],
                                     xh[:, 64 * c0:64 * c0 + 128], id128,
                                     is_transpose=True,
                                     start=(ci == 0), stop=(ci == 1))
                # psT partition (cb, d): cb selects odd/even c
                for cb in range(2):
                    nc.vector.tensor_copy(
                        tgt[:, b4, cb],
                        psT[64 * cb:64 * cb + 64])

    def project(st):
        for nm in ("q", "k", "v"):
            for ch in range(2):
                project_unit(st, nm, ch, ptag="S")

    _qkt_cache = {}

    def _qkt_of(pair):
        qkt = _qkt_cache.get(pair)
        if qkt is None:
            qkt = ptile.tile([128, 2 * S], F16, tag="QKT", bufs=2,
                             name=f"QKT{pair}_{rep}")
            _qkt_cache[pair] = qkt
        return qkt

    def attend_load(pair):
        QKT = _qkt_of(pair)
        return QKT[:, 0:S], QKT[:, S:2 * S]

    def attend(pair, loaded, fillers=(), tail_fill=((), ()),
               vp_lazy=False):
        QT, KT = loaded
        fillers = list(fillers)
        vps = [None, None]

        def load_vp(half):
            h = 2 * pair + half
            vp = ptile.tile([128, 16, 65], F16, tag="vp", bufs=4,
                            name=f"vp{h}_{rep}")
            v_src = bass.AP(vh_d.tensor, vh_d.offset + 128 * h * E,
                            [[64, 128], [8192, 16], [1, 64]])
            with tc.high_priority():
                nc.sync.dma_start(out=vp[:, :, 0:64], in_=v_src)
            nc.vector.memset(vp[:, :, 64:65], 1.0)
            vps[half] = vp

        if not vp_lazy:
            load_vp(0)
            load_vp(1)

        LAG = 5   # defer V-matmuls 5 groups behind S^T/exp (pt bufs cover it)
        pending = []
        deferred = []   # finish_b closures, run 1-2 steps after their qc ends

        def emit_vmms(ent):
            # P^T@V with tall [128q, 65] outputs: per (j, qsub) block the PE
            # charge is 65 cols instead of 512 (M/K are free in the model).
            qc_, js_, pts_, po_ = ent
            for half, pt in pts_.items():
                if vps[half] is None:
                    load_vp(half)
                for jj, j in enumerate(js_):
                    for qs in range(4):
                        if 4 * qc_ + qs < j:
                            continue
                        # single psum group per (half, qc) bank: start only
                        # on the very first block, stop on the very last
                        nc.tensor.matmul(
                            po_[half][:, qs, :],
                            pt[:, 512 * jj + 128 * qs:
                               512 * jj + 128 * qs + 128],
                            vps[half][:, j, :],
                            start=(j == 0 and qs == 0),
                            stop=(j == 4 * qc_ + 3 and qs == 3))

        def finish_a(half, qc, po):
            # DVE part: rowsums -> reciprocals -> normalized [q, d] tiles.
            # Frees the po psum buffer; PE transposes come later (finish_b)
            # so the in-order PE stream never waits on this chain.
            h = 2 * pair + half
            rec = ptile.tile([128, 4, 1], F32, tag="rec", bufs=2,
                             name=f"rec{h}{qc}_{rep}")
            nc.vector.reciprocal(rec, po[:, :, 64:65])
            pn = ptile.tile([128, 4, 64], F16, tag="pn", bufs=2,
                            name=f"pn{h}{qc}_{rep}")
            # one TT with a stride-0 broadcast of rec along d
            rec_b = bass.AP(rec.tensor, rec.offset,
                            [list(d) for d in rec.ap[:2]] + [[0, 64]])
            nc.vector.tensor_tensor(pn, po[:, :, 0:64], rec_b,
                                    op=mybir.AluOpType.mult)
            return pn

        def finish_b(half, qc, pn):
            # PE transposes [128q,64d] -> [64d,128q], then strided copies
            # into the final-projection layout xT2.
            h = 2 * pair + half
            po2 = ps.tile([128, 2, 128], F16, tag="T", bufs=1,
                          name=f"po2{h}{qc}_{rep}")
            for qp in range(2):
                nc.tensor.matmul(po2[:, qp], pn[:, 2 * qp:2 * qp + 2], id128,
                                 is_transpose=True,
                                 start=(qp == 0), stop=(qp == 1))
            # po2[64qb+d, qp, ql]: query q = 128*(2qp+qb) + ql,
            # ql = 16rq + 2c2 + h2 -> xT2[64h2+d, c2, 32qc+16qp+8qb+rq]
            p_re = po2.rearrange("p qp (rq c2 h2) -> p qp h2 c2 rq",
                                 c2=8, h2=2)
            x_re = xT2[:, :, 128 * h + 32 * qc:
                       128 * h + 32 * qc + 32].rearrange(
                "p t (qp qb2 rq) -> p qp qb2 t rq", qp=2, qb2=2)
            for qb in range(2):
                for h2 in range(2):
                    nc.vector.tensor_copy(
                        x_re[64 * h2:64 * h2 + 64, :, qb],
                        p_re[64 * qb:64 * qb + 64, :, h2])
            if qc == 3:
                for f in tail_fill[half]:
                    f()

        def pop_pending():
            ent = pending.pop(0)
            emit_vmms(ent)
            qc_, js_ = ent[0], ent[1]
            if js_[-1] == 4 * qc_ + 3:   # last group of its qc
                for half in ent[2]:
                    pn = finish_a(half, qc_, ent[3][half])
                    deferred.append(
                        lambda half=half, qc_=qc_, pn=pn:
                        finish_b(half, qc_, pn))

        def emit_half_step(qc, js, half):
            psS = ps.tile([128, 1024], F32, tag="S", bufs=2,
                          name=f"psS{half}_{qc}_{js[0]}_{rep}")
            pt = ptile.tile([128, 1024], F16, tag="P", bufs=12,
                            name=f"pt{half}_{qc}_{js[0]}_{rep}")
            r0, r1 = 64 * half, 64 * half + 64
            exp_runs = []    # (lo, hi) spans to exp, exact width
            for jj, j in enumerate(js):
                o = j - 4 * qc
                lo = 0 if o < 0 else 128 * o
                nc.tensor.matmul(
                    psS[:, 512 * jj + lo:512 * jj + 512],
                    KT[r0:r1, bass.ts(j, 128)],
                    QT[r0:r1, 512 * qc + lo:512 * qc + 512],
                    start=True, stop=True)
                lo_, hi_ = 512 * jj + lo, 512 * jj + 512
                if exp_runs and exp_runs[-1][1] == lo_:
                    exp_runs[-1] = (exp_runs[-1][0], hi_)
                else:
                    exp_runs.append((lo_, hi_))
            for lo_, hi_ in exp_runs:
                nc.scalar.activation(pt[:, lo_:hi_], psS[:, lo_:hi_], EXP)
            for jj, j in enumerate(js):
                o = j - 4 * qc
                if o >= 0:
                    sl = pt[:, 512 * jj + 128 * o:512 * jj + 128 * o + 128]
                    nc.gpsimd.affine_select(
                        out=sl, in_=sl,
                        pattern=[[1, 128]],
                        compare_op=mybir.AluOpType.is_ge,
                        fill=0.0, base=0, channel_multiplier=-1)
            return pt

        def step_tail():
            if fillers:
                fillers.pop(0)()   # independent PE work while exp runs
            if deferred:
                deferred.pop(0)()
            if len(pending) > LAG:
                pop_pending()

        po_of = {}
        for qc in (1, 0, 2, 3):
            jmax = 4 * qc + 3
            po_of[qc] = [ps.tile([128, 4, 65], F32, tag="O", bufs=2,
                                 name=f"po{2 * pair + half}_{qc}_{rep}")
                         for half in range(2)]
            if qc != 3 or pair == 0:
                for j0 in range(0, jmax + 1, 2):
                    js = [j0, j0 + 1]
                    pts = {half: emit_half_step(qc, js, half)
                           for half in range(2)}
                    pending.append((qc, js, pts, po_of[qc]))
                    step_tail()
            else:
                # last chunk: halves staggered so the first head's finish +
                # final projection overlap the second head's exps
                for half in range(2):
                    for j0 in range(0, jmax + 1, 2):
                        js = [j0, j0 + 1]
                        pts = {half: emit_half_step(qc, js, half)}
                        pending.append((qc, js, pts, po_of[qc]))
                        step_tail()
        # drain. Leftover deferred work is handed to the caller (it becomes
        # the next pair's first fillers) so the PE stream never parks on the
        # finish chain at a pair boundary.
        while pending:
            pop_pending()
        if tail_fill != ((), ()):
            while deferred:
                deferred.pop(0)()
        for f in fillers:
            f()
        return deferred

    def final_unit(st, ch, ptag="P1"):
            # tail units borrow the psS slots (attention is over by then)
            py = ps.tile([128, 512] if ptag == "P1" else [128, 1024], F32,
                         tag=ptag, bufs=1 if ptag == "P1" else 2,
                         name=f"py{st}{ch}_{rep}")[:, 0:512]
            for t in range(8):
                nc.tensor.matmul(py,
                                 xT2[:, t, bass.ts(st, 128)],
                                 w_sb["p"][:, t, bass.ts(ch, 512)],
                                 start=(t == 0), stop=(t == 7))
            ysb = ypool.tile([128, 512], F16, tag="y",
                             name=f"ysb{st}{ch}_{rep}")
            nc.vector.tensor_tensor(ysb, py,
                                    bias_sb["p"][:, bass.ts(ch, 512)],
                                    op=mybir.AluOpType.add)
            nc.sync.dma_start(out=y[bass.ts(st, 128), bass.ts(ch, 512)],
                              in_=ysb)

    def final(st):
        for ch in range(2):
            final_unit(st, ch)

    # pipeline: proj st0/st1 dense; pair-0 attention with proj st2/st3 as
    # PE fillers; pair-1 attention with final st0/st1 as fillers; tail.
    _xh_cache.clear()
    if phases < 2:
        if load_inputs is not None:
            load_inputs("v")
        for st in range(4):
            project(st)
        return
    def _load_p():
        # demote the final-projection weight stream: it must never win the
        # DMA device or a sem slot from attention-critical transfers
        with tc.high_priority(offset=-(10 ** 6)):
            load_inputs("p")

    # q/k projections of tiles 0/1 first so pair-0 transposes start early.
    # Everything else (v, q/k st2/3) goes to fillers so the in-order PE
    # stream reaches the first scores as soon as KT(pair0) is built.
    for st, nm in ((0, "q"), (1, "q"), (0, "k"), (1, "k")):
        for ch in range(2):
            project_unit(st, nm, ch, ptag="S")
    loaded0 = attend_load(0)
    if load_inputs is not None:
        load_inputs("v")
    loaded1_box = {}
    fill0 = [
        (lambda st=st, ch=ch: project_unit(st, "v", ch))
        for st in (0, 1) for ch in range(2)
    ] + [
        (lambda st=st, nm=nm, ch=ch: project_unit(st, nm, ch))
        for nm in ("q", "k") for st in (2, 3) for ch in range(2)
    ] + [
        lambda: loaded1_box.update(v=attend_load(1)),
        (lambda: _load_p()) if load_inputs is not None else (lambda: None),
    ]
    left0 = attend(0, loaded0, fill0, vp_lazy=True)
    loaded1 = loaded1_box["v"]
    if phases >= 3:
        # v st2/3 projections run as pair-1's first fillers: pair 0 is
        # PE-bound while pair 1 has PE slack, and the lazily-loaded V tiles
        # aren't consumed until the PV lag expires (~5 steps in)
        fill1 = [
            (lambda st=st, ch=ch: project_unit(st, "v", ch))
            for st in (2, 3) for ch in range(2)
        ] + list(left0) + [
            (lambda st=st, ch=ch: final_unit(st, ch))
            for st in (0, 1) for ch in range(2)
        ]
        tails = ([(lambda ch=ch: final_unit(2, ch, ptag="S"))
                  for ch in range(2)],
                 [(lambda ch=ch: final_unit(3, ch, ptag="S"))
                  for ch in range(2)])
    else:
        fill1, tails = [], ((), ())
    attend(1, loaded1, fill1, tail_fill=tails, vp_lazy=True)


# ---------------------------------------------------------------------------
# host side
# ---------------------------------------------------------------------------

_CACHE = {}


def _prep_inputs(q, k, v, Wq, bq, Wk, bk, Wv, bv, Wp, bp):
    scale = 1.0 / np.sqrt(64.0)
    wq_T = np.ascontiguousarray(np.asarray(Wq, np.float32).T).astype(np.float16)
    wk_T = np.ascontiguousarray(np.asarray(Wk, np.float32).T).astype(np.float16)
    wv_T = np.ascontiguousarray(np.asarray(Wv, np.float32).T).astype(np.float16)
    wp_T = np.ascontiguousarray(np.asarray(Wp, np.float32).T).astype(np.float16)
    shared = {
        "wqT": wq_T, "wkT": wk_T, "wvT": wv_T, "wpT": wp_T,
        "bq": (np.asarray(bq, np.float32) * scale).astype(np.float16)[None, :],
        "bk": np.asarray(bk, np.float16)[None, :],
        "bv": np.asarray(bv, np.float16)[None, :],
        "bp": np.asarray(bp, np.float16)[None, :],
    }
    in_maps = []
    for c in range(N_CORES):
        b, g = divmod(c, 4)
        rows = slice(SB * g, SB * (g + 1))
        m = dict(shared)
        m["qT"] = np.ascontiguousarray(
            np.asarray(q[b, rows], np.float32).T * scale).astype(np.float16)
        m["kT"] = np.ascontiguousarray(
            np.asarray(k[b, rows], np.float32).T).astype(np.float16)
        m["vT"] = np.ascontiguousarray(
            np.asarray(v[b, rows], np.float32).T).astype(np.float16)
        in_maps.append(m)
    return in_maps


def kernel(q, k, v, Wq, bq, Wk, bk, Wv, bv, Wp, bp):
    if "nc" not in _CACHE:
        _CACHE["nc"] = build()
    nc = _CACHE["nc"]
    in_maps = _prep_inputs(q, k, v, Wq, bq, Wk, bk, Wv, bv, Wp, bp)
    res = run_bass_kernel_spmd(nc, in_maps, core_ids=list(range(N_CORES)))
    out = np.empty((B, S, E), np.float32)
    for c in range(N_CORES):
        b, g = divmod(c, 4)
        out[b, SB * g:SB * (g + 1), :] = res.results[c]["y"]
    return out


if __name__ == "__main__":
    rng = np.random.default_rng(0)
    s = 1.0 / np.sqrt(E)
    ins = {
        "q": rng.standard_normal((B, S, E), dtype=np.float32),
        "k": rng.standard_normal((B, S, E), dtype=np.float32),
        "v": rng.standard_normal((B, S, E), dtype=np.float32),
        "Wq": rng.standard_normal((E, E), dtype=np.float32) * s,
        "bq": rng.standard_normal(E).astype(np.float32) * s,
        "Wk": rng.standard_normal((E, E), dtype=np.float32) * s,
        "bk": rng.standard_normal(E).astype(np.float32) * s,
        "Wv": rng.standard_normal((E, E), dtype=np.float32) * s,
        "bv": rng.standard_normal(E).astype(np.float32) * s,
        "Wp": rng.standard_normal((E, E), dtype=np.float32) * s,
        "bp": rng.standard_normal(E).astype(np.float32) * s,
    }
    out = kernel(**ins)
    print("kernel ran, out shape", out.shape, "mean", float(np.abs(out).mean()))

